# revision 8
# baseline (speedup 1.0000x reference)
"""GPTBigCode transformer block (MQA) on 8 trn2 NeuronCores — v2.

Sharding: data-parallel over batch (4) x parity-interleaved q-block split
(2) per batch element. Core c handles batch c//2 and q-blocks {2j + c%2}.
No collectives; K/V (single MQA head) recomputed per core.

v2 keeps ALL activations feature-on-partition ("T layout") end-to-end —
zero PE transposes. LayerNorm statistics are computed with ones-vector
matmul chains (partition-axis reduction on the tensor engine), per-token
scalars are broadcast back across partitions with K=1 matmuls. Attention
computes transposed scores (keys-on-partition) so softmax-denominators
come from ones-matmuls and probs feed attn@V directly. The softmax
normalization is applied as a per-column multiply on the attention
output. Causal masking of the parity-dependent diagonal zone uses two
per-core mask inputs so the compiled program is identical on all cores.

Weights are host-packed so every weight DMA is contiguous per partition
line; activations never round-trip through DRAM. Matmul inputs bf16;
accumulation, softmax and residual math f32 (residual stream bf16).
"""

import numpy as np
import ml_dtypes

# ---------------------------------------------------------------------------
# Workaround: this container's walrus build rejects >1 sync-wait on
# CTRL-class (Drain) instructions. Split the Tile tail-drain's waits into
# individual wait-carrying NOPs on the SP engine.
import bass_rust
from concourse.tile import TileContext
from concourse.vector_clock import ScopedClock


def _patched_drain_and_barrier(self, tick_clock, wait_clock):
    nc = self.nc
    drain_inst = nc.sync.drain()
    wait_clock.add_sem_waits(
        drain_inst.ins, ScopedClock({None: tick_clock.global_clock})
    )
    si = drain_inst.ins.sync_info
    waits = list(si.on_wait) if si and si.on_wait else []
    if len(waits) > 1:
        drain_inst.ins.sync_info = bass_rust.SyncInfo(
            on_wait=waits[:1],
            on_update=list(si.on_update) if si.on_update else [],
        )
        for w in waits[1:]:
            n = nc.sync.nop(nofuse=True, hint="split_drain_wait")
            n.ins.sync_info = bass_rust.SyncInfo(on_wait=[w], on_update=[])
    nc.all_engine_barrier()
    assert self.sems is not None
    popped = nc._tile_sem_poison_stack.pop()
    assert popped is self._sem_poison
    nc.clear_and_free_semaphores(list(self.sems.allocated().values()))
    nc.all_engine_barrier()


TileContext._drain_and_barrier = _patched_drain_and_barrier


def _split_excess_waits(nc, max_waits=1):
    """Rewrite every instruction carrying more than `max_waits` sem-waits:
    excess waits move onto same-engine NOPs inserted just before it."""
    all_bbs = [bb for fn in nc.m.functions for bb in fn.blocks]
    for bb in all_bbs:
        insts = list(bb.instructions)
        new_list = []
        changed = False
        for inst in insts:
            si = inst.sync_info
            waits = list(si.on_wait) if si and si.on_wait else []
            if len(waits) > max_waits:
                changed = True
                inst.sync_info = bass_rust.SyncInfo(
                    on_wait=waits[:max_waits],
                    on_update=list(si.on_update) if si.on_update else [],
                )
                for w in waits[max_waits:]:
                    nop_bi = nc.engines[inst.engine].nop(
                        nofuse=True, hint="wsplit"
                    )
                    nop = nop_bi.ins
                    cur = nc.cur_bb.bb
                    cl = list(cur.instructions)
                    assert cl and cl[-1].name == nop.name, "nop not appended last"
                    cur.instructions = cl[:-1]
                    nop.sync_info = bass_rust.SyncInfo(on_wait=[w], on_update=[])
                    new_list.append(nop)
            new_list.append(inst)
        if changed:
            bb.instructions = new_list
# ---------------------------------------------------------------------------

import concourse.bass as bass
import concourse.mybir as mybir
from concourse.bass_utils import run_bass_kernel_spmd
from concourse.masks import make_identity

f32 = mybir.dt.float32
bf16 = mybir.dt.bfloat16
AF = mybir.ActivationFunctionType
ALU = mybir.AluOpType

H = 2048
NH = 16
D = 128
INTER = 8192
S = 2048
B = 4
NQ = 1024          # query tokens per core
HT = H // 128      # 16
IT = INTER // 128  # 64
NCH = S // 512     # 4 full-seq chunks
EPS = 1e-5
NEG = -30000.0
INV_H = 1.0 / H


def _ln_rows(nc, rowp, workp2, psB, sum_ps, sumsq_ps, eps_t):
    """[1,512] psum sums -> (m_sb f32, rstd_sb f32) row tiles."""
    m_sb = rowp.tile([1, 512], f32, tag="rows", bufs=2, name="m_sb")
    nc.scalar.mul(m_sb, sum_ps, INV_H)
    v_sb = rowp.tile([1, 512], f32, tag="rows", bufs=2, name="v_sb")
    nc.scalar.mul(v_sb, sumsq_ps, INV_H)
    m2 = workp2.tile([128, 512], f32, tag="t2k", bufs=2, name="m2")
    nc.vector.tensor_mul(m2[0:1, :], m_sb, m_sb)
    nc.vector.tensor_sub(v_sb, v_sb, m2[0:1, :])
    # rstd = exp(-0.5*ln(var+eps)) — keeps the whole row path on ScalarE
    # (DVE reciprocal on a 1-partition row is ~3.3us serial); both steps
    # in place so the rows tag needs only 2 slots.
    nc.scalar.activation(v_sb, v_sb, AF.Ln, bias=eps_t)
    nc.scalar.activation(v_sb, v_sb, AF.Exp, scale=-0.5)
    return m_sb, v_sb


def _ln_chunk_stats(nc, workp, rowp, psB, psC, xc, ones_col, eps_t):
    """LN stats for one [128,16,512] bf16 chunk (raw x, T layout). sum and
    sumsq live in different psum tags so consecutive chunks' stats chains
    double-buffer instead of serializing on one tag pair."""
    sum_ps = psB.tile([1, 512], f32, tag="B", bufs=2, name="sum_ps")
    for kt in range(HT):
        nc.tensor.matmul(sum_ps, ones_col, xc[:, kt, :],
                         start=(kt == 0), stop=(kt == HT - 1))
    # sumsq: squares on ACT, elementwise accumulate on DVE (runs parallel
    # to the PE sum chain), single ones-matmul reduction.
    sqacc = workp.tile([128, 512], bf16, tag="exac", bufs=2, name="sqa")
    nc.scalar.square(sqacc, xc[:, 0, :])
    for kt in range(1, HT):
        sq = workp.tile([128, 512], bf16, tag="t2k", bufs=2, name="sq")
        nc.scalar.square(sq, xc[:, kt, :])
        nc.vector.tensor_add(sqacc, sqacc, sq)
    sumsq_ps = psC.tile([1, 512], f32, tag="C", bufs=2, name="sumsq_ps")
    nc.tensor.matmul(sumsq_ps, ones_col, sqacc, start=True, stop=True)
    return _ln_rows(nc, rowp, workp, psB, sum_ps, sumsq_ps, eps_t)


def _bcast_row_bf16(nc, workp, psC, ones_rowf, row_sb, name):
    """[1,512] f32 row -> [128,512] bf16 sbuf broadcast tile."""
    bc_ps = psC.tile([128, 512], f32, tag="C", bufs=2, name=f"{name}_ps")
    nc.tensor.matmul(bc_ps, ones_rowf, row_sb, start=True, stop=True)
    bc_sb = workp.tile([128, 512], bf16, tag="bc1k", bufs=2, name=f"{name}_sb")
    nc.scalar.copy(bc_sb, bc_ps)
    return bc_sb


def _scale_inplace(nc, xc, bcr):
    """xc *= rstd (per column). The mean is folded into the following
    matmul chains as a K=1 rank-1 correction with -colsum(W)."""
    for kt in range(HT):
        nc.vector.tensor_mul(xc[:, kt, :], xc[:, kt, :], bcr)


def _build_program():
    nc = bass.Bass(trn_type="TRN2")

    xt_d = nc.dram_tensor("xt", [NCH, 128, HT, 512], bf16, kind="ExternalInput")
    xq_d = nc.dram_tensor("xq", [2, 128, HT, 512], bf16, kind="ExternalInput")
    xtq_d = nc.dram_tensor("xtq", [HT, 2, 128, 512], bf16, kind="ExternalInput")
    wq_d = nc.dram_tensor("wq", [NH, 128, HT, 128], bf16, kind="ExternalInput")
    wk_d = nc.dram_tensor("wk", [128, HT, 128], bf16, kind="ExternalInput")
    wv_d = nc.dram_tensor("wv", [128, HT, 128], bf16, kind="ExternalInput")
    wo_d = nc.dram_tensor("wo", [HT, 128, HT, 128], bf16, kind="ExternalInput")
    wfc_d = nc.dram_tensor("wfc", [IT, 128, HT, 128], bf16, kind="ExternalInput")
    wproj_d = nc.dram_tensor("wproj", [HT, 128, IT, 128], bf16, kind="ExternalInput")
    bq_d = nc.dram_tensor("bq", [128, NH], f32, kind="ExternalInput")
    bk_d = nc.dram_tensor("bk", [128, 1], f32, kind="ExternalInput")
    bv_d = nc.dram_tensor("bv", [1, 128], bf16, kind="ExternalInput")
    bo_d = nc.dram_tensor("bo", [128, HT], f32, kind="ExternalInput")
    bfc_d = nc.dram_tensor("bfc", [128, IT], f32, kind="ExternalInput")
    bproj_d = nc.dram_tensor("bproj", [128, HT], f32, kind="ExternalInput")
    maskA_d = nc.dram_tensor("maskA", [128, 512], bf16, kind="ExternalInput")
    maskB_d = nc.dram_tensor("maskB", [128, 512], bf16, kind="ExternalInput")
    csq_d = nc.dram_tensor("csq", [1, NH, 128], bf16, kind="ExternalInput")
    csk_d = nc.dram_tensor("csk", [1, 128], bf16, kind="ExternalInput")
    csv_d = nc.dram_tensor("csv", [1, 128], bf16, kind="ExternalInput")
    out_d = nc.dram_tensor("out", [H, NQ], f32, kind="ExternalOutput")

    with TileContext(nc) as tc:
        with (
            tc.tile_pool(name="const", bufs=1) as constp,
            tc.tile_pool(name="big", bufs=1) as bigp,
            tc.tile_pool(name="s32", bufs=2) as s32p,
            tc.tile_pool(name="str16", bufs=2) as strp,
            tc.tile_pool(name="band", bufs=3) as bandp,
            tc.tile_pool(name="work", bufs=2) as workp,
            tc.tile_pool(name="rows", bufs=4) as rowp,
            tc.tile_pool(name="psA", bufs=2, space="PSUM") as psA,
            tc.tile_pool(name="psB", bufs=2, space="PSUM") as psB,
            tc.tile_pool(name="psC", bufs=2, space="PSUM") as psC,
        ):
            # All four full-seq chunks prefetch into one big tile that
            # tag-shares with gT (xc_all is dead before the MLP writes gT,
            # so both fit in one 64KB/partition slot). Chunk 0 is split in
            # two DMAs so compute can start on its first half early.
            xc_all = bigp.tile([128, NCH, HT, 512], bf16, tag="big",
                               name="xc_all")
            nc.sync.dma_start(xc_all[:, 0, 0:8, :], xt_d[0][:, 0:8, :])
            nc.sync.dma_start(xc_all[:, 0, 8:16, :], xt_d[0][:, 8:16, :])
            for ci in range(1, NCH):
                nc.sync.dma_start(xc_all[:, ci], xt_d[ci])

            # ---- constants ----
            ones_col = constp.tile([128, 1], bf16, name="ones_col")
            nc.vector.memset(ones_col, 1.0)
            ones_rowf = constp.tile([1, 128], f32, name="ones_rowf")
            nc.vector.memset(ones_rowf, 1.0)
            ones_rowb = constp.tile([1, 128], bf16, name="ones_rowb")
            nc.vector.memset(ones_rowb, 1.0)
            eps_t = constp.tile([1, 1], f32, name="eps_t")
            nc.vector.memset(eps_t, EPS)
            bq_sb = constp.tile([128, NH], f32, name="bq_sb")
            nc.sync.dma_start(bq_sb, bq_d[:, :])
            bk_sb = constp.tile([128, 1], f32, name="bk_sb")
            nc.sync.dma_start(bk_sb, bk_d[:, :])
            bv_sb = constp.tile([1, 128], bf16, name="bv_sb")
            nc.sync.dma_start(bv_sb, bv_d[:, :])
            bo_sb = constp.tile([128, HT], f32, name="bo_sb")
            nc.sync.dma_start(bo_sb, bo_d[:, :])
            bfc_sb = constp.tile([128, IT], f32, name="bfc_sb")
            nc.sync.dma_start(bfc_sb, bfc_d[:, :])
            bproj_sb = constp.tile([128, HT], f32, name="bproj_sb")
            nc.sync.dma_start(bproj_sb, bproj_d[:, :])
            maskA = constp.tile([128, 512], bf16, name="maskA")
            nc.sync.dma_start(maskA, maskA_d[:, :])
            maskB = constp.tile([128, 512], bf16, name="maskB")
            nc.sync.dma_start(maskB, maskB_d[:, :])
            csq_sb = constp.tile([1, NH, 128], bf16, name="csq_sb")
            nc.sync.dma_start(csq_sb, csq_d[:, :, :])
            csk_sb = constp.tile([1, 128], bf16, name="csk_sb")
            nc.sync.dma_start(csk_sb, csk_d[:, :])
            csv_sb = constp.tile([1, 128], bf16, name="csv_sb")
            nc.sync.dma_start(csv_sb, csv_d[:, :])
            kT_sb = constp.tile([128, S], bf16, name="kT_sb")
            vtm = constp.tile([128, HT, 128], bf16, name="vtm")
            # K/V weights are tiny (512KB each) — load once into two band
            # slots; they stay live across all four chunks of phase A.
            wk_sb = bandp.tile([128, HT, 128], bf16, tag="band4", bufs=3,
                               name="wk_sb")
            nc.sync.dma_start(wk_sb, wk_d[:, :, :])
            wv_sb = bandp.tile([128, HT, 128], bf16, tag="band4", bufs=3,
                               name="wv_sb")
            nc.sync.dma_start(wv_sb, wv_d[:, :, :])

            # ---- phase A: full-seq LN1 + K/V, streamed in 512-token chunks
            for ci in range(NCH):
                xc = xc_all[:, ci]
                m_sb, rstd = _ln_chunk_stats(nc, workp, rowp, psB, psC, xc,
                                             ones_col, eps_t)
                bcr = _bcast_row_bf16(nc, workp, psC, ones_rowf, rstd, "bcr")
                mr = workp.tile([1, 512], bf16, tag="xq1k", bufs=2, name="mr")
                nc.vector.tensor_mul(mr, m_sb, rstd)
                _scale_inplace(nc, xc, bcr)
                # K^T chunk: [dk=128, 512 tokens]
                kps = psA.tile([128, 2, 512], f32, tag="A", bufs=2, name="kps")
                for kt in range(HT):
                    nc.tensor.matmul(kps[:, 0, :], wk_sb[:, kt, :], xc[:, kt, :],
                                     start=(kt == 0), stop=False)
                nc.tensor.matmul(kps[:, 0, :], csk_sb, mr,
                                 start=False, stop=True)
                nc.scalar.activation(kT_sb[:, ci * 512:(ci + 1) * 512],
                                     kps[:, 0, :], AF.Identity, bias=bk_sb)
                # V token-major: 4 token-blocks
                for tb in range(4):
                    vps = psA.tile([128, 2, 512], f32, tag="A", bufs=2, name="vps")
                    for kt in range(HT):
                        nc.tensor.matmul(
                            vps[:, 0, 0:128],
                            xc[:, kt, tb * 128:(tb + 1) * 128],
                            wv_sb[:, kt, :], start=(kt == 0), stop=False,
                        )
                    nc.tensor.matmul(vps[:, 0, 0:128], ones_rowb, bv_sb,
                                     start=False, stop=False)
                    nc.tensor.matmul(vps[:, 0, 0:128],
                                     mr[:, tb * 128:(tb + 1) * 128], csv_sb,
                                     start=False, stop=True)
                    nc.vector.tensor_copy(vtm[:, ci * 4 + tb, :], vps[:, 0, 0:128])

            # ---- phase B: own-q LN1 + Q projection (2 chunks of 512) ----
            # qT layout [dq, hg, j, hh, q]: scores rhs [:, hg, j] is a fully
            # contiguous 512-column block (4 heads x 128 q).
            qT = s32p.tile([128, 4, 8, 4, 128], bf16, tag="s32", name="qT")
            for ch in range(2):
                xqc = strp.tile([128, HT, 512], bf16, tag="str16", name="xqc")
                nc.sync.dma_start(xqc, xq_d[ch])
                m_sb, rstd = _ln_chunk_stats(nc, workp, rowp, psB, psC, xqc,
                                             ones_col, eps_t)
                bcr = _bcast_row_bf16(nc, workp, psC, ones_rowf, rstd, "qbcr")
                mr = workp.tile([1, 512], bf16, tag="xq1k", bufs=2, name="qmr")
                nc.vector.tensor_mul(mr, m_sb, rstd)
                _scale_inplace(nc, xqc, bcr)
                for m in range(NH):
                    hg, hh = divmod(m, 4)
                    band = bandp.tile([128, HT, 128], bf16, tag="band4",
                                      bufs=3, name="band")
                    nc.sync.dma_start(band, wq_d[m])
                    qps = psA.tile([128, 2, 512], f32, tag="A", bufs=2, name="qps")
                    for kt in range(HT):
                        nc.tensor.matmul(qps[:, 0, :], band[:, kt, :],
                                         xqc[:, kt, :],
                                         start=(kt == 0), stop=False)
                    nc.tensor.matmul(qps[:, 0, :], csq_sb[:, m, :], mr,
                                     start=False, stop=True)
                    nc.scalar.activation(qT[:, hg, 4 * ch:4 * ch + 4, hh, :],
                                         qps[:, 0, :], AF.Identity,
                                         bias=bq_sb[:, m:m + 1])

            # ---- phase C: attention (scoresT, padded extent E=2j+2) ----
            # Two q-blocks' ladders are interleaved per head-group so the PE
            # fills the ACT-exp latency of one block with the other block's
            # matmuls.
            attnT = s32p.tile([128, NH, NQ], bf16, tag="s32", name="attnT")

            def _attn_step(hg, j, p0, kts, exacc, av_ps):
                """One 2-kt step of block j: two scores matmuls into a 2-bank
                psum tile, one exp over both, av accumulation on the PE and
                elementwise exp accumulation on the (otherwise idle) DVE —
                the softmax denominator then needs only ONE ones-matmul per
                block instead of one per k-tile. Causal masking is a 0/1
                multiply on exp(s) (exp(s+M) = exp(s)·exp(M)) — on the DVE,
                keeping the PE free of mask matmuls."""
                E = len(kts)
                sc = psA.tile([128, 2, 512], f32, tag="A", bufs=2, name="sc")
                for dk in range(2):
                    kt = kts[p0 + dk]
                    nc.tensor.matmul(
                        sc[:, dk, :], kT_sb[:, kt * 128:(kt + 1) * 128],
                        qT[:, hg, j], start=True, stop=True,
                    )
                ex = workp.tile([128, 2, 512], bf16, tag="t2k", bufs=2,
                                name="ex")
                nc.scalar.activation(ex, sc, AF.Exp)
                if p0 == 0:
                    # kts[0], kts[1] are the parity-dependent diagonal tiles.
                    nc.vector.tensor_mul(ex[:, 0, :], ex[:, 0, :], maskA)
                    nc.vector.tensor_mul(ex[:, 1, :], ex[:, 1, :], maskB)
                    nc.vector.tensor_copy(exacc, ex[:, 0, :])
                else:
                    nc.vector.tensor_add(exacc, exacc, ex[:, 0, :])
                nc.vector.tensor_add(exacc, exacc, ex[:, 1, :])
                for dk in range(2):
                    idx = p0 + dk
                    kt = kts[idx]
                    nc.tensor.matmul(av_ps, vtm[:, kt, :], ex[:, dk, :],
                                     start=(idx == 0), stop=(idx == E - 1))

            def _attn_tail(hg, j, exacc, av_ps):
                den_ps = psB.tile([1, 512], f32, tag="B", bufs=2,
                                  name="den_ps")
                nc.tensor.matmul(den_ps, ones_col, exacc, start=True, stop=True)
                lnd = rowp.tile([1, 512], f32, tag="rows", bufs=2, name="lnd")
                nc.scalar.activation(lnd, den_ps, AF.Ln)
                nc.scalar.activation(lnd, lnd, AF.Exp, scale=-1.0)
                rec = lnd
                bcr_ps = psB.tile([128, 512], f32, tag="B", bufs=2,
                                  name="bcr_ps")
                nc.tensor.matmul(bcr_ps, ones_rowf, rec, start=True, stop=True)
                bcr_sb = workp.tile([128, 512], bf16, tag="bc1k", bufs=2,
                                    name="bcr_sb")
                # DVE copy: ACT is the bottleneck engine in this phase.
                nc.vector.tensor_copy(bcr_sb, bcr_ps)
                nc.vector.tensor_mul(
                    attnT[:, hg * 4:(hg + 1) * 4, j * 128:(j + 1) * 128],
                    av_ps, bcr_sb,
                )

            for hg in range(4):
                for jp in range(4):
                    j0, j1 = 2 * jp, 2 * jp + 1
                    E0, E1 = 2 * j0 + 2, 2 * j1 + 2
                    kts0 = [E0 - 2, E0 - 1] + list(range(E0 - 2))
                    kts1 = [E1 - 2, E1 - 1] + list(range(E1 - 2))
                    ea0 = workp.tile([128, 512], bf16, tag="exac", bufs=2,
                                     name="ea0")
                    av0 = psC.tile([128, 512], f32, tag="C", bufs=2, name="av0")
                    ea1 = workp.tile([128, 512], bf16, tag="exac", bufs=2,
                                     name="ea1")
                    av1 = psC.tile([128, 512], f32, tag="C", bufs=2, name="av1")
                    for p0 in range(0, E1, 2):
                        if p0 < E0:
                            _attn_step(hg, j0, p0, kts0, ea0, av0)
                        elif p0 == E0:
                            _attn_tail(hg, j0, ea0, av0)
                        _attn_step(hg, j1, p0, kts1, ea1, av1)
                    _attn_tail(hg, j1, ea1, av1)

            # ---- phase D: out-proj + residual -> y ----
            # ch-outer: y[:, :, ch0] completes early so LN2+fc of chunk 0
            # overlap the second wo half.
            y = s32p.tile([128, HT, NQ], bf16, tag="s32", name="y")

            def _ln2_acc(ch):
                """LN2 elementwise stats accumulation for one chunk — pure
                ACT/DVE work, emitted right after the chunk's y completes so
                it hides under the next PE-dense stretch."""
                cols = slice(ch * 512, (ch + 1) * 512)
                yacc = workp.tile([128, 512], bf16, tag="exac", bufs=2,
                                  name="yacc")
                nc.vector.tensor_copy(yacc, y[:, 0, cols])
                sqacc = workp.tile([128, 512], bf16, tag="exac", bufs=2,
                                   name="sqacc")
                nc.scalar.square(sqacc, y[:, 0, cols])
                for kt in range(1, HT):
                    nc.vector.tensor_add(yacc, yacc, y[:, kt, cols])
                    sq = workp.tile([128, 512], bf16, tag="t2k", bufs=2,
                                    name="sq2")
                    nc.scalar.square(sq, y[:, kt, cols])
                    nc.vector.tensor_add(sqacc, sqacc, sq)
                return yacc, sqacc

            def _ln2_fin(ch, yacc, sqacc):
                """LN2 reduction matmuls + broadcast + normalize."""
                cols = slice(ch * 512, (ch + 1) * 512)
                sum_ps = psB.tile([1, 512], f32, tag="B", bufs=2, name="l2sum")
                nc.tensor.matmul(sum_ps, ones_col, yacc, start=True, stop=True)
                sumsq_ps = psC.tile([1, 512], f32, tag="C", bufs=2, name="l2sq")
                nc.tensor.matmul(sumsq_ps, ones_col, sqacc, start=True,
                                 stop=True)
                m_sb, rstd = _ln_rows(nc, rowp, workp, psB, sum_ps, sumsq_ps,
                                      eps_t)
                bcm = _bcast_row_bf16(nc, workp, psC, ones_rowf, m_sb, "l2bcm")
                bcr = _bcast_row_bf16(nc, workp, psC, ones_rowf, rstd, "l2bcr")
                ln2s = s32p.tile([128, HT, 512], bf16, tag="s32", name="ln2s")
                for kt in range(HT):
                    nc.vector.tensor_sub(ln2s[:, kt, :], y[:, kt, cols], bcm)
                    nc.vector.tensor_mul(ln2s[:, kt, :], ln2s[:, kt, :], bcr)
                return ln2s

            ln2_accs = [None, None]
            for ch in range(2):
                for ob in range(HT):
                    band = bandp.tile([128, HT, 128], bf16, tag="band4", bufs=3,
                                      name="band")
                    nc.sync.dma_start(band, wo_d[ob])
                    wps = psA.tile([128, 2, 512], f32, tag="A", bufs=2,
                                   name="wps")
                    for ht in range(HT):
                        nc.tensor.matmul(wps[:, 0, :], band[:, ht, :],
                                         attnT[:, ht, ch * 512:(ch + 1) * 512],
                                         start=(ht == 0), stop=(ht == HT - 1))
                    xqt = workp.tile([128, 512], bf16, tag="xq1k", bufs=2,
                                     name="xqt")
                    nc.sync.dma_start(xqt, xtq_d[ob, ch])
                    nc.vector.scalar_tensor_tensor(
                        out=y[:, ob, ch * 512:(ch + 1) * 512],
                        in0=wps[:, 0, :], scalar=bo_sb[:, ob:ob + 1],
                        in1=xqt, op0=ALU.add, op1=ALU.add,
                    )
                ln2_accs[ch] = _ln2_acc(ch)

            # ---- phase E: LN2 + MLP + residual -> out (per 512-token chunk)
            for ch in range(2):
                cols = slice(ch * 512, (ch + 1) * 512)
                ln2s = _ln2_fin(ch, *ln2_accs[ch])

                gT = bigp.tile([128, IT, 512], bf16, tag="big", name="gT")
                for mb in range(IT):
                    band = bandp.tile([128, HT, 128], bf16, tag="band4",
                                      bufs=3, name="band")
                    nc.sync.dma_start(band, wfc_d[mb])
                    fps = psA.tile([128, 2, 512], f32, tag="A", bufs=2,
                                   name="fps")
                    for kt in range(HT):
                        nc.tensor.matmul(fps[:, 0, :], band[:, kt, :],
                                         ln2s[:, kt, :],
                                         start=(kt == 0), stop=(kt == HT - 1))
                    nc.scalar.activation(gT[:, mb, :], fps[:, 0, :],
                                         AF.Gelu_apprx_tanh,
                                         bias=bfc_sb[:, mb:mb + 1])

                for ob in range(HT):
                    pband = strp.tile([128, IT, 128], bf16, tag="str16",
                                      name="pband")
                    nc.sync.dma_start(pband, wproj_d[ob])
                    pps = psA.tile([128, 2, 512], f32, tag="A", bufs=2,
                                   name="pps")
                    for mt in range(IT):
                        nc.tensor.matmul(pps[:, 0, :], pband[:, mt, :],
                                         gT[:, mt, :],
                                         start=(mt == 0), stop=(mt == IT - 1))
                    osb = workp.tile([128, 512], f32, tag="f2k", bufs=2,
                                     name="osb")
                    nc.vector.scalar_tensor_tensor(
                        out=osb, in0=pps[:, 0, :],
                        scalar=bproj_sb[:, ob:ob + 1],
                        in1=y[:, ob, cols], op0=ALU.add, op1=ALU.add,
                    )
                    nc.sync.dma_start(
                        out_d[ob * 128:(ob + 1) * 128, cols], osb
                    )
    _split_excess_waits(nc)
    return nc


_PROG = None


def _get_prog():
    global _PROG
    if _PROG is None:
        _PROG = _build_program()
    return _PROG


def _to_bf(a):
    return np.ascontiguousarray(a.astype(ml_dtypes.bfloat16))


def kernel(hidden_states, ln1_g, ln1_b, ln2_g, ln2_b, wq, bq, wkv, bkv,
           wo, bo, wfc, bfc, wproj, bproj):
    hs = np.asarray(hidden_states, np.float32)
    ln1_g = np.asarray(ln1_g, np.float32)
    ln1_b = np.asarray(ln1_b, np.float32)
    ln2_g = np.asarray(ln2_g, np.float32)
    ln2_b = np.asarray(ln2_b, np.float32)
    wq = np.asarray(wq, np.float32)
    wkv = np.asarray(wkv, np.float32)
    wo = np.asarray(wo, np.float32)
    wfc = np.asarray(wfc, np.float32)
    wproj = np.asarray(wproj, np.float32)

    # Fold LN gains into the following matmuls; fold qk scale into K.
    wq_e = ln1_g[:, None] * wq
    bq_e = np.asarray(bq, np.float32) + ln1_b @ wq
    wkv_e = ln1_g[:, None] * wkv
    bkv_e = np.asarray(bkv, np.float32) + ln1_b @ wkv
    scale = 1.0 / np.sqrt(D)
    wk_e = wkv_e[:, :D] * scale
    bk_e = bkv_e[:D] * scale
    wv_e = wkv_e[:, D:]
    bv_e = bkv_e[D:]
    wfc_e = ln2_g[:, None] * wfc
    bfc_e = np.asarray(bfc, np.float32) + ln2_b @ wfc

    # Host-packed weight layouts: [out-block, partition, k-tile, n] so each
    # band DMA is contiguous per partition line.
    wq_l = _to_bf(wq_e.reshape(HT, 128, NH, 128).transpose(2, 1, 0, 3))
    wk_l = _to_bf(wk_e.reshape(HT, 128, 128).transpose(1, 0, 2))
    wv_l = _to_bf(wv_e.reshape(HT, 128, 128).transpose(1, 0, 2))
    wo_l = _to_bf(wo.reshape(HT, 128, HT, 128).transpose(2, 1, 0, 3))
    wfc_l = _to_bf(wfc_e.reshape(HT, 128, IT, 128).transpose(2, 1, 0, 3))
    wproj_l = _to_bf(wproj.reshape(IT, 128, HT, 128).transpose(2, 1, 0, 3))

    # Negated column sums for the K=1 LN-mean-fold correction matmuls.
    csq_r = _to_bf(-wq_e.sum(axis=0).reshape(1, NH, 128))
    csk_r = _to_bf(-wk_e.sum(axis=0)[None, :])
    csv_r = _to_bf(-wv_e.sum(axis=0)[None, :])

    bq_r = np.ascontiguousarray(bq_e.reshape(NH, 128).T)
    bk_r = np.ascontiguousarray(bk_e[:, None])
    bv_r = _to_bf(bv_e[None, :])
    bo_r = np.ascontiguousarray(np.asarray(bo, np.float32).reshape(HT, 128).T)
    bfc_r = np.ascontiguousarray(bfc_e.reshape(IT, 128).T)
    bproj_r = np.ascontiguousarray(
        np.asarray(bproj, np.float32).reshape(HT, 128).T)

    # Multiplicative 0/1 causal masks for the two parity-dependent diagonal
    # k-tiles (applied to exp(s) on the DVE: exp(s+M) = exp(s)·1[M==0]).
    tri = np.where(np.arange(128)[None, :] >= np.arange(128)[:, None],
                   1.0, 0.0).astype(np.float32)          # [k,q] keep-mask
    tri4 = np.tile(tri, (1, 4))                          # [128, 512] (4 heads)
    zeros4 = np.zeros((128, 512), np.float32)
    ones4 = np.ones((128, 512), np.float32)
    mask_h = [(_to_bf(tri4), _to_bf(zeros4)),            # parity 0: (A, B)
              (_to_bf(ones4), _to_bf(tri4))]             # parity 1: (A, B)

    in_maps = []
    gmaps = []
    for c in range(8):
        b, h = divmod(c, 2)
        gmap = [2 * j + h for j in range(8)]
        gmaps.append(gmap)
        xb = hs[b]                                        # [2048, 2048]
        xt_h = _to_bf(xb.reshape(NCH, 512, HT, 128).transpose(0, 3, 2, 1))
        xqb = xb.reshape(16, 128, H)[gmap].reshape(NQ, H)  # [1024, 2048]
        xq_h = _to_bf(xqb.reshape(2, 512, HT, 128).transpose(0, 3, 2, 1))
        xtq_h = _to_bf(xqb.reshape(2, 512, HT, 128).transpose(2, 0, 3, 1))
        mA, mB = mask_h[h]
        in_maps.append(dict(
            xt=xt_h, xq=xq_h, xtq=xtq_h,
            wq=wq_l, wk=wk_l, wv=wv_l, wo=wo_l, wfc=wfc_l, wproj=wproj_l,
            bq=bq_r, bk=bk_r, bv=bv_r, bo=bo_r, bfc=bfc_r, bproj=bproj_r,
            maskA=mA, maskB=mB, csq=csq_r, csk=csk_r, csv=csv_r,
        ))

    res = run_bass_kernel_spmd(_get_prog(), in_maps, core_ids=list(range(8)))
    kernel.last_result = res

    out = np.empty((B, S, H), np.float32)
    for c in range(8):
        b, h = divmod(c, 2)
        resT = np.asarray(res.results[c]["out"])          # [2048, 1024]
        blocks = resT.T.reshape(8, 128, H)                # local q-blocks
        for j, g in enumerate(gmaps[c]):
            out[b, g * 128:(g + 1) * 128, :] = blocks[j]
    return out


kernel.last_result = None



# revision 25
# speedup vs baseline: 1.0451x; 1.0451x over previous
"""GPTBigCode transformer block (MQA) on 8 trn2 NeuronCores — v2.

Sharding: data-parallel over batch (4) x parity-interleaved q-block split
(2) per batch element. Core c handles batch c//2 and q-blocks {2j + c%2}.
No collectives; K/V (single MQA head) recomputed per core.

v2 keeps ALL activations feature-on-partition ("T layout") end-to-end —
zero PE transposes. LayerNorm statistics are computed with ones-vector
matmul chains (partition-axis reduction on the tensor engine), per-token
scalars are broadcast back across partitions with K=1 matmuls. Attention
computes transposed scores (keys-on-partition) so softmax-denominators
come from ones-matmuls and probs feed attn@V directly. The softmax
normalization is applied as a per-column multiply on the attention
output. Causal masking of the parity-dependent diagonal zone uses two
per-core mask inputs so the compiled program is identical on all cores.

Weights are host-packed so every weight DMA is contiguous per partition
line; activations never round-trip through DRAM. Matmul inputs bf16;
accumulation, softmax and residual math f32 (residual stream bf16).
"""

import numpy as np
import ml_dtypes

# ---------------------------------------------------------------------------
# Workaround: this container's walrus build rejects >1 sync-wait on
# CTRL-class (Drain) instructions. Split the Tile tail-drain's waits into
# individual wait-carrying NOPs on the SP engine.
import bass_rust
from concourse.tile import TileContext
from concourse.vector_clock import ScopedClock


def _patched_drain_and_barrier(self, tick_clock, wait_clock):
    nc = self.nc
    drain_inst = nc.sync.drain()
    wait_clock.add_sem_waits(
        drain_inst.ins, ScopedClock({None: tick_clock.global_clock})
    )
    si = drain_inst.ins.sync_info
    waits = list(si.on_wait) if si and si.on_wait else []
    if len(waits) > 1:
        drain_inst.ins.sync_info = bass_rust.SyncInfo(
            on_wait=waits[:1],
            on_update=list(si.on_update) if si.on_update else [],
        )
        for w in waits[1:]:
            n = nc.sync.nop(nofuse=True, hint="split_drain_wait")
            n.ins.sync_info = bass_rust.SyncInfo(on_wait=[w], on_update=[])
    nc.all_engine_barrier()
    assert self.sems is not None
    popped = nc._tile_sem_poison_stack.pop()
    assert popped is self._sem_poison
    nc.clear_and_free_semaphores(list(self.sems.allocated().values()))
    nc.all_engine_barrier()


TileContext._drain_and_barrier = _patched_drain_and_barrier


def _split_excess_waits(nc, max_waits=1):
    """Rewrite every instruction carrying more than `max_waits` sem-waits:
    excess waits move onto same-engine NOPs inserted just before it."""
    all_bbs = [bb for fn in nc.m.functions for bb in fn.blocks]
    for bb in all_bbs:
        insts = list(bb.instructions)
        new_list = []
        changed = False
        for inst in insts:
            si = inst.sync_info
            waits = list(si.on_wait) if si and si.on_wait else []
            if len(waits) > max_waits:
                changed = True
                inst.sync_info = bass_rust.SyncInfo(
                    on_wait=waits[:max_waits],
                    on_update=list(si.on_update) if si.on_update else [],
                )
                for w in waits[max_waits:]:
                    nop_bi = nc.engines[inst.engine].nop(
                        nofuse=True, hint="wsplit"
                    )
                    nop = nop_bi.ins
                    cur = nc.cur_bb.bb
                    cl = list(cur.instructions)
                    assert cl and cl[-1].name == nop.name, "nop not appended last"
                    cur.instructions = cl[:-1]
                    nop.sync_info = bass_rust.SyncInfo(on_wait=[w], on_update=[])
                    new_list.append(nop)
            new_list.append(inst)
        if changed:
            bb.instructions = new_list
# ---------------------------------------------------------------------------

import concourse.bass as bass
import concourse.mybir as mybir
from concourse.bass_utils import run_bass_kernel_spmd
from concourse.masks import make_identity

f32 = mybir.dt.float32
bf16 = mybir.dt.bfloat16
AF = mybir.ActivationFunctionType
ALU = mybir.AluOpType

H = 2048
NH = 16
D = 128
INTER = 8192
S = 2048
B = 4
NQ = 1024          # query tokens per core
HT = H // 128      # 16
IT = INTER // 128  # 64
NCH = S // 512     # 4 full-seq chunks
EPS = 1e-5
NEG = -30000.0
INV_H = 1.0 / H


def _ln_rows(nc, rowp, workp2, psB, sum_ps, sumsq_ps, eps_t):
    """[1,512] psum sums -> (m_sb f32, rstd_sb f32) row tiles."""
    m_sb = rowp.tile([1, 512], f32, tag="rows", bufs=2, name="m_sb")
    nc.scalar.mul(m_sb, sum_ps, INV_H)
    v_sb = rowp.tile([1, 512], f32, tag="rows", bufs=2, name="v_sb")
    nc.scalar.mul(v_sb, sumsq_ps, INV_H)
    m2 = workp2.tile([128, 512], f32, tag="t2k", bufs=2, name="m2")
    nc.vector.tensor_mul(m2[0:1, :], m_sb, m_sb)
    nc.vector.tensor_sub(v_sb, v_sb, m2[0:1, :])
    # rstd = exp(-0.5*ln(var+eps)) — keeps the whole row path on ScalarE
    # (DVE reciprocal on a 1-partition row is ~3.3us serial); both steps
    # in place so the rows tag needs only 2 slots.
    nc.scalar.activation(v_sb, v_sb, AF.Ln, bias=eps_t)
    nc.scalar.activation(v_sb, v_sb, AF.Exp, scale=-0.5)
    return m_sb, v_sb


def _ln_chunk_stats(nc, workp, rowp, psB, psC, xc, ones_col, eps_t):
    """LN stats for one [128,16,512] bf16 chunk (raw x, T layout). sum and
    sumsq live in different psum tags so consecutive chunks' stats chains
    double-buffer instead of serializing on one tag pair. Squares are
    batched two k-tiles per ACT op to halve the serial ACT chain that
    otherwise gates the chunk."""
    sum_ps = psB.tile([1, 512], f32, tag="B", bufs=2, name="sum_ps")
    for kt in range(HT):
        nc.tensor.matmul(sum_ps, ones_col, xc[:, kt, :],
                         start=(kt == 0), stop=(kt == HT - 1))
    sqacc = workp.tile([128, 2, 512], bf16, tag="exac", bufs=2, name="sqa")
    nc.scalar.square(sqacc, xc[:, 0:2, :])
    for kt in range(2, HT, 2):
        sq = workp.tile([128, 2, 512], bf16, tag="t2k", bufs=2, name="sq")
        nc.scalar.square(sq, xc[:, kt:kt + 2, :])
        nc.vector.tensor_add(sqacc, sqacc, sq)
    nc.vector.tensor_add(sqacc[:, 0, :], sqacc[:, 0, :], sqacc[:, 1, :])
    sumsq_ps = psC.tile([1, 512], f32, tag="C", bufs=2, name="sumsq_ps")
    nc.tensor.matmul(sumsq_ps, ones_col, sqacc[:, 0, :], start=True, stop=True)
    return _ln_rows(nc, rowp, workp, psB, sum_ps, sumsq_ps, eps_t)


def _bcast_row_bf16(nc, workp, psC, ones_rowf, row_sb, name):
    """[1,512] f32 row -> [128,512] bf16 sbuf broadcast tile."""
    bc_ps = psC.tile([128, 512], f32, tag="C", bufs=2, name=f"{name}_ps")
    nc.tensor.matmul(bc_ps, ones_rowf, row_sb, start=True, stop=True)
    bc_sb = workp.tile([128, 512], bf16, tag="bc1k", bufs=2, name=f"{name}_sb")
    nc.scalar.copy(bc_sb, bc_ps)
    return bc_sb


# LN1 is folded into the projections: out = rstd ∘ (W^T x_raw − m·csW) + b.
# x stays RAW in SBUF (no in-place scale); the mean term is a K=1 rank-1
# correction matmul with -colsum(W) against the plain mean row, and the rstd
# scale is a single per-output-tile DVE multiply with the broadcast tile.
# The K bias is dropped entirely (softmax-invariant) and the V bias is
# folded into bo host-side (attention probs sum to 1).


def _build_program():
    nc = bass.Bass(trn_type="TRN2")

    xt_d = nc.dram_tensor("xt", [NCH, 128, HT, 512], bf16, kind="ExternalInput")
    xq_d = nc.dram_tensor("xq", [2, 128, HT, 512], bf16, kind="ExternalInput")
    xtq_d = nc.dram_tensor("xtq", [HT, 2, 128, 512], bf16, kind="ExternalInput")
    wq_d = nc.dram_tensor("wq", [NH, 128, HT, 128], bf16, kind="ExternalInput")
    wk_d = nc.dram_tensor("wk", [128, HT, 128], bf16, kind="ExternalInput")
    wv_d = nc.dram_tensor("wv", [128, HT, 128], bf16, kind="ExternalInput")
    wo_d = nc.dram_tensor("wo", [HT, 128, HT, 128], bf16, kind="ExternalInput")
    wfc_d = nc.dram_tensor("wfc", [IT, 128, HT, 128], bf16, kind="ExternalInput")
    wproj_d = nc.dram_tensor("wproj", [HT, 128, IT, 128], bf16, kind="ExternalInput")
    bq_d = nc.dram_tensor("bq", [128, NH], f32, kind="ExternalInput")
    bo_d = nc.dram_tensor("bo", [128, HT], f32, kind="ExternalInput")
    bfc_d = nc.dram_tensor("bfc", [128, IT], f32, kind="ExternalInput")
    bproj_d = nc.dram_tensor("bproj", [128, HT], f32, kind="ExternalInput")
    maskA_d = nc.dram_tensor("maskA", [128, 512], bf16, kind="ExternalInput")
    maskB_d = nc.dram_tensor("maskB", [128, 512], bf16, kind="ExternalInput")
    csq_d = nc.dram_tensor("csq", [1, NH, 128], bf16, kind="ExternalInput")
    csk_d = nc.dram_tensor("csk", [1, 128], bf16, kind="ExternalInput")
    csv_d = nc.dram_tensor("csv", [1, 128], bf16, kind="ExternalInput")
    out_d = nc.dram_tensor("out", [H, NQ], bf16, kind="ExternalOutput")

    with TileContext(nc) as tc:
        with (
            tc.tile_pool(name="const", bufs=1) as constp,
            tc.tile_pool(name="big", bufs=1) as bigp,
            tc.tile_pool(name="s32", bufs=2) as s32p,
            tc.tile_pool(name="str16", bufs=2) as strp,
            tc.tile_pool(name="band", bufs=3) as bandp,
            tc.tile_pool(name="work", bufs=2) as workp,
            tc.tile_pool(name="rows", bufs=4) as rowp,
            tc.tile_pool(name="psA", bufs=2, space="PSUM") as psA,
            tc.tile_pool(name="psB", bufs=2, space="PSUM") as psB,
            tc.tile_pool(name="psC", bufs=2, space="PSUM") as psC,
        ):
            # All four full-seq chunks prefetch into one big tile that
            # tag-shares with gT (xc_all is dead before the MLP writes gT,
            # so both fit in one 64KB/partition slot). Chunk 0 is split in
            # two DMAs so compute can start on its first half early.
            xc_all = bigp.tile([128, NCH, HT, 512], bf16, tag="big",
                               name="xc_all")
            nc.sync.dma_start(xc_all[:, 0, 0:8, :], xt_d[0][:, 0:8, :])
            nc.sync.dma_start(xc_all[:, 0, 8:16, :], xt_d[0][:, 8:16, :])
            for ci in range(1, NCH):
                nc.sync.dma_start(xc_all[:, ci], xt_d[ci])

            # ---- constants ----
            ones_col = constp.tile([128, 1], bf16, name="ones_col")
            nc.vector.memset(ones_col, 1.0)
            ones_rowf = constp.tile([1, 128], f32, name="ones_rowf")
            nc.vector.memset(ones_rowf, 1.0)
            eps_t = constp.tile([1, 1], f32, name="eps_t")
            nc.vector.memset(eps_t, EPS)
            bq_sb = constp.tile([128, NH], f32, name="bq_sb")
            nc.sync.dma_start(bq_sb, bq_d[:, :])
            bo_sb = constp.tile([128, HT], f32, name="bo_sb")
            nc.sync.dma_start(bo_sb, bo_d[:, :])
            bfc_sb = constp.tile([128, IT], f32, name="bfc_sb")
            nc.sync.dma_start(bfc_sb, bfc_d[:, :])
            bproj_sb = constp.tile([128, HT], f32, name="bproj_sb")
            nc.sync.dma_start(bproj_sb, bproj_d[:, :])
            maskA = constp.tile([128, 512], bf16, name="maskA")
            nc.sync.dma_start(maskA, maskA_d[:, :])
            maskB = constp.tile([128, 512], bf16, name="maskB")
            nc.sync.dma_start(maskB, maskB_d[:, :])
            csq_sb = constp.tile([1, NH, 128], bf16, name="csq_sb")
            nc.sync.dma_start(csq_sb, csq_d[:, :, :])
            csk_sb = constp.tile([1, 128], bf16, name="csk_sb")
            nc.sync.dma_start(csk_sb, csk_d[:, :])
            csv_sb = constp.tile([1, 128], bf16, name="csv_sb")
            nc.sync.dma_start(csv_sb, csv_d[:, :])
            id_bf = constp.tile([128, 128], bf16, name="id_bf")
            make_identity(nc, id_bf)
            kT_sb = constp.tile([128, S], bf16, name="kT_sb")
            vtm = constp.tile([128, HT, 128], bf16, name="vtm")
            # K/V weights are tiny (512KB each) — load once into two band
            # slots; they stay live across all four chunks of phase A.
            wk_sb = bandp.tile([128, HT, 128], bf16, tag="band4", bufs=3,
                               name="wk_sb")
            nc.sync.dma_start(wk_sb, wk_d[:, :, :])
            wv_sb = bandp.tile([128, HT, 128], bf16, tag="band4", bufs=3,
                               name="wv_sb")
            nc.sync.dma_start(wv_sb, wv_d[:, :, :])
            # Own-q chunks prefetch early too.
            xqcs = []
            for ch in range(2):
                xqc = strp.tile([128, HT, 512], bf16, tag="str16", name="xqc")
                nc.sync.dma_start(xqc, xq_d[ch])
                xqcs.append(xqc)

            # ---- phase A/B: LN1 stats + K/V/Q projections ----
            # Stats for chunk ci+1 are emitted before chunk ci's projections
            # so the serial square/add chain of the next chunk hides under
            # the current chunk's PE-dense projection work.

            def _stats(xc, name):
                m_sb, rstd = _ln_chunk_stats(nc, workp, rowp, psB, psC, xc,
                                             ones_col, eps_t)
                bcr = _bcast_row_bf16(nc, workp, psC, ones_rowf, rstd, name)
                mrow = workp.tile([1, 512], bf16, tag="xq1k", bufs=2,
                                  name="mrow")
                nc.vector.tensor_copy(mrow, m_sb)
                return bcr, mrow

            def _kv_chunk(ci, bcr, mrow):
                xc = xc_all[:, ci]
                # K^T chunk: [dk=128, 512 tokens]; K bias dropped (softmax-
                # invariant), rstd applied on the output tile.
                kps = psA.tile([128, 2, 512], f32, tag="A", bufs=2, name="kps")
                for kt in range(HT):
                    nc.tensor.matmul(kps[:, 0, :], wk_sb[:, kt, :],
                                     xc[:, kt, :], start=(kt == 0), stop=False)
                nc.tensor.matmul(kps[:, 0, :], csk_sb, mrow,
                                 start=False, stop=True)
                nc.vector.tensor_mul(kT_sb[:, ci * 512:(ci + 1) * 512],
                                     kps[:, 0, :], bcr)
                # V feature-major [dv, 512], then four PE transposes into the
                # token-major vtm the AV matmuls need. V bias folded into bo.
                vps = psA.tile([128, 2, 512], f32, tag="A", bufs=2, name="vps")
                for kt in range(HT):
                    nc.tensor.matmul(vps[:, 0, :], wv_sb[:, kt, :],
                                     xc[:, kt, :], start=(kt == 0), stop=False)
                nc.tensor.matmul(vps[:, 0, :], csv_sb, mrow,
                                 start=False, stop=True)
                vfm = workp.tile([128, 512], bf16, tag="vfm", bufs=2,
                                 name="vfm")
                nc.vector.tensor_mul(vfm, vps[:, 0, :], bcr)
                for tb in range(4):
                    tp = psB.tile([128, 128], bf16, tag="B", bufs=2, name="tp")
                    nc.tensor.transpose(tp, vfm[:, tb * 128:(tb + 1) * 128],
                                        id_bf)
                    nc.vector.tensor_copy(vtm[:, ci * 4 + tb, :], tp)

            def _q_half(ch, bcr, mrow):
                xqc = xqcs[ch]
                for m in range(NH):
                    hg, hh = divmod(m, 4)
                    band = bandp.tile([128, HT, 128], bf16, tag="band4",
                                      bufs=3, name="band")
                    nc.sync.dma_start(band, wq_d[m])
                    qps = psA.tile([128, 2, 512], f32, tag="A", bufs=2,
                                   name="qps")
                    for kt in range(HT):
                        nc.tensor.matmul(qps[:, 0, :], band[:, kt, :],
                                         xqc[:, kt, :],
                                         start=(kt == 0), stop=False)
                    nc.tensor.matmul(qps[:, 0, :], csq_sb[:, m, :], mrow,
                                     start=False, stop=True)
                    qsc = workp.tile([128, 512], bf16, tag="vfm", bufs=2,
                                     name="qsc")
                    nc.vector.tensor_mul(qsc, qps[:, 0, :], bcr)
                    nc.scalar.activation(qT[:, hg, 4 * ch:4 * ch + 4, hh, :],
                                         qsc, AF.Identity,
                                         bias=bq_sb[:, m:m + 1])

            # qT layout [dq, hg, j, hh, q]: scores rhs [:, hg, j] is a fully
            # contiguous 512-column block (4 heads x 128 q).
            qT = s32p.tile([128, 4, 8, 4, 128], bf16, tag="s32", name="qT")

            st = [None] * 6   # 4 full chunks + 2 q-halves
            st[0] = _stats(xc_all[:, 0], "bcr0")
            st[1] = _stats(xc_all[:, 1], "bcr1")
            _kv_chunk(0, *st[0])
            st[2] = _stats(xc_all[:, 2], "bcr2")
            _kv_chunk(1, *st[1])
            st[3] = _stats(xc_all[:, 3], "bcr3")
            _kv_chunk(2, *st[2])
            st[4] = _stats(xqcs[0], "bcrq0")
            _kv_chunk(3, *st[3])
            st[5] = _stats(xqcs[1], "bcrq1")
            _q_half(0, *st[4])
            _q_half(1, *st[5])

            # ---- phase C: attention (scoresT, padded extent E=2j+2) ----
            # Two q-blocks' ladders are interleaved per head-group so the PE
            # fills the ACT-exp latency of one block with the other block's
            # matmuls.
            attnT = s32p.tile([128, NH, NQ], bf16, tag="s32", name="attnT")

            def _attn_step(hg, j, p0, kts, exacc, av_ps):
                """One 2-kt step of block j: two scores matmuls into a 2-bank
                psum tile, one exp over both, av accumulation on the PE and
                elementwise exp accumulation on the (otherwise idle) DVE —
                the softmax denominator then needs only ONE ones-matmul per
                block instead of one per k-tile. Causal masking is a 0/1
                multiply on exp(s) (exp(s+M) = exp(s)·exp(M)) — on the DVE,
                keeping the PE free of mask matmuls."""
                E = len(kts)
                sc = psA.tile([128, 2, 512], f32, tag="A", bufs=2, name="sc")
                for dk in range(2):
                    kt = kts[p0 + dk]
                    masked = kt >= E - 2
                    nc.tensor.matmul(
                        sc[:, dk, :], kT_sb[:, kt * 128:(kt + 1) * 128],
                        qT[:, hg, j], start=True, stop=not masked,
                    )
                    if masked:
                        # mask add on the PE: sc += I^T @ mask. Keeping this
                        # inside the accumulation group avoids a cross-engine
                        # hop on the exp->AV critical path (measured: a DVE
                        # 0/1-multiply here cost ~1us/block in AV stalls).
                        nc.tensor.matmul(sc[:, dk, :], id_bf,
                                         maskA if kt == E - 2 else maskB,
                                         start=False, stop=True)
                ex = workp.tile([128, 2, 512], bf16, tag="t2k", bufs=2,
                                name="ex")
                nc.scalar.activation(ex, sc, AF.Exp)
                if p0 == 0:
                    nc.vector.tensor_copy(exacc, ex[:, 0, :])
                else:
                    nc.vector.tensor_add(exacc, exacc, ex[:, 0, :])
                nc.vector.tensor_add(exacc, exacc, ex[:, 1, :])
                for dk in range(2):
                    idx = p0 + dk
                    kt = kts[idx]
                    nc.tensor.matmul(av_ps, vtm[:, kt, :], ex[:, dk, :],
                                     start=(idx == 0), stop=(idx == E - 1))

            def _attn_tail(hg, j, exacc, av_ps):
                den_ps = psB.tile([1, 512], f32, tag="B", bufs=2,
                                  name="den_ps")
                nc.tensor.matmul(den_ps, ones_col, exacc, start=True, stop=True)
                lnd = rowp.tile([1, 512], f32, tag="rows", bufs=2, name="lnd")
                nc.scalar.activation(lnd, den_ps, AF.Ln)
                nc.scalar.activation(lnd, lnd, AF.Exp, scale=-1.0)
                rec = lnd
                bcr_ps = psB.tile([128, 512], f32, tag="B", bufs=2,
                                  name="bcr_ps")
                nc.tensor.matmul(bcr_ps, ones_rowf, rec, start=True, stop=True)
                bcr_sb = workp.tile([128, 512], bf16, tag="bc1k", bufs=2,
                                    name="bcr_sb")
                # DVE copy: ACT is the bottleneck engine in this phase.
                nc.vector.tensor_copy(bcr_sb, bcr_ps)
                nc.vector.tensor_mul(
                    attnT[:, hg * 4:(hg + 1) * 4, j * 128:(j + 1) * 128],
                    av_ps, bcr_sb,
                )

            for hg in range(4):
                for jp in range(4):
                    j0, j1 = 2 * jp, 2 * jp + 1
                    E0, E1 = 2 * j0 + 2, 2 * j1 + 2
                    kts0 = [E0 - 2, E0 - 1] + list(range(E0 - 2))
                    kts1 = [E1 - 2, E1 - 1] + list(range(E1 - 2))
                    ea0 = workp.tile([128, 512], bf16, tag="exac", bufs=2,
                                     name="ea0")
                    av0 = psC.tile([128, 512], f32, tag="C", bufs=2, name="av0")
                    ea1 = workp.tile([128, 512], bf16, tag="exac", bufs=2,
                                     name="ea1")
                    av1 = psC.tile([128, 512], f32, tag="C", bufs=2, name="av1")
                    for p0 in range(0, E1, 2):
                        if p0 < E0:
                            _attn_step(hg, j0, p0, kts0, ea0, av0)
                        elif p0 == E0:
                            _attn_tail(hg, j0, ea0, av0)
                        _attn_step(hg, j1, p0, kts1, ea1, av1)
                    _attn_tail(hg, j1, ea1, av1)

            # ---- phase D: out-proj + residual -> y ----
            # ch-outer: y[:, :, ch0] completes early so LN2+fc of chunk 0
            # overlap the second wo half.
            y = s32p.tile([128, HT, NQ], bf16, tag="s32", name="y")

            def _ln2_acc(ch):
                """LN2 elementwise stats accumulation for one chunk — pure
                ACT/DVE work (two k-tiles per op), emitted right after the
                chunk's y completes so it hides under PE-dense stretches."""
                cols = slice(ch * 512, (ch + 1) * 512)
                yacc = workp.tile([128, 2, 512], bf16, tag="exac", bufs=2,
                                  name="yacc")
                nc.vector.tensor_add(yacc, y[:, 0:2, cols], y[:, 2:4, cols])
                for kt in range(4, HT, 2):
                    nc.vector.tensor_add(yacc, yacc, y[:, kt:kt + 2, cols])
                nc.vector.tensor_add(yacc[:, 0, :], yacc[:, 0, :],
                                     yacc[:, 1, :])
                sqacc = workp.tile([128, 2, 512], bf16, tag="exac", bufs=2,
                                   name="sqacc")
                nc.scalar.square(sqacc, y[:, 0:2, cols])
                for kt in range(2, HT, 2):
                    sq = workp.tile([128, 2, 512], bf16, tag="t2k", bufs=2,
                                    name="sq2")
                    nc.scalar.square(sq, y[:, kt:kt + 2, cols])
                    nc.vector.tensor_add(sqacc, sqacc, sq)
                nc.vector.tensor_add(sqacc[:, 0, :], sqacc[:, 0, :],
                                     sqacc[:, 1, :])
                return yacc, sqacc

            def _ln2_fin(ch, yacc, sqacc):
                """LN2 reduction matmuls + broadcast + normalize."""
                cols = slice(ch * 512, (ch + 1) * 512)
                sum_ps = psB.tile([1, 512], f32, tag="B", bufs=2, name="l2sum")
                nc.tensor.matmul(sum_ps, ones_col, yacc[:, 0, :], start=True,
                                 stop=True)
                sumsq_ps = psC.tile([1, 512], f32, tag="C", bufs=2, name="l2sq")
                nc.tensor.matmul(sumsq_ps, ones_col, sqacc[:, 0, :],
                                 start=True, stop=True)
                m_sb, rstd = _ln_rows(nc, rowp, workp, psB, sum_ps, sumsq_ps,
                                      eps_t)
                bcm = _bcast_row_bf16(nc, workp, psC, ones_rowf, m_sb, "l2bcm")
                bcr = _bcast_row_bf16(nc, workp, psC, ones_rowf, rstd, "l2bcr")
                ln2s = s32p.tile([128, HT, 512], bf16, tag="s32", name="ln2s")
                for kt in range(HT):
                    nc.vector.tensor_sub(ln2s[:, kt, :], y[:, kt, cols], bcm)
                    nc.vector.tensor_mul(ln2s[:, kt, :], ln2s[:, kt, :], bcr)
                return ln2s

            for ch in range(2):
                for ob in range(HT):
                    band = bandp.tile([128, HT, 128], bf16, tag="band4", bufs=3,
                                      name="band")
                    nc.sync.dma_start(band, wo_d[ob])
                    wps = psA.tile([128, 2, 512], f32, tag="A", bufs=2,
                                   name="wps")
                    for ht in range(HT):
                        nc.tensor.matmul(wps[:, 0, :], band[:, ht, :],
                                         attnT[:, ht, ch * 512:(ch + 1) * 512],
                                         start=(ht == 0), stop=(ht == HT - 1))
                    xqt = workp.tile([128, 512], bf16, tag="xq1k", bufs=2,
                                     name="xqt")
                    nc.sync.dma_start(xqt, xtq_d[ob, ch])
                    nc.vector.scalar_tensor_tensor(
                        out=y[:, ob, ch * 512:(ch + 1) * 512],
                        in0=wps[:, 0, :], scalar=bo_sb[:, ob:ob + 1],
                        in1=xqt, op0=ALU.add, op1=ALU.add,
                    )
                if ch == 0:
                    acc0 = _ln2_acc(0)

            # ---- phase E: LN2 + MLP + residual -> out (per 512-token chunk)
            # Emission order keeps each chunk's stats chain off the critical
            # path: fin0 right after D (acc0 ran under D-ch1), acc1 under
            # fc-ch0, fin1 between fc-ch0 and proj-ch0.
            ln2s_pair = [_ln2_fin(0, *acc0), None]
            acc1 = _ln2_acc(1)
            for ch in range(2):
                cols = slice(ch * 512, (ch + 1) * 512)
                ln2s = ln2s_pair[ch]

                gT = bigp.tile([128, IT, 512], bf16, tag="big", name="gT")
                for mb in range(IT):
                    band = bandp.tile([128, HT, 128], bf16, tag="band4",
                                      bufs=3, name="band")
                    nc.sync.dma_start(band, wfc_d[mb])
                    fps = psA.tile([128, 2, 512], f32, tag="A", bufs=2,
                                   name="fps")
                    for kt in range(HT):
                        nc.tensor.matmul(fps[:, 0, :], band[:, kt, :],
                                         ln2s[:, kt, :],
                                         start=(kt == 0), stop=(kt == HT - 1))
                    nc.scalar.activation(gT[:, mb, :], fps[:, 0, :],
                                         AF.Gelu_apprx_tanh,
                                         bias=bfc_sb[:, mb:mb + 1])
                if ch == 0:
                    ln2s_pair[1] = _ln2_fin(1, *acc1)

                for ob in range(HT):
                    pband = strp.tile([128, IT, 128], bf16, tag="str16",
                                      name="pband")
                    nc.sync.dma_start(pband, wproj_d[ob])
                    pps = psA.tile([128, 2, 512], f32, tag="A", bufs=2,
                                   name="pps")
                    for mt in range(IT):
                        nc.tensor.matmul(pps[:, 0, :], pband[:, mt, :],
                                         gT[:, mt, :],
                                         start=(mt == 0), stop=(mt == IT - 1))
                    osb = workp.tile([128, 512], bf16, tag="f2k", bufs=2,
                                     name="osb")
                    nc.vector.scalar_tensor_tensor(
                        out=osb, in0=pps[:, 0, :],
                        scalar=bproj_sb[:, ob:ob + 1],
                        in1=y[:, ob, cols], op0=ALU.add, op1=ALU.add,
                    )
                    nc.sync.dma_start(
                        out_d[ob * 128:(ob + 1) * 128, cols], osb
                    )
    _split_excess_waits(nc)
    return nc


_PROG = None


def _get_prog():
    global _PROG
    if _PROG is None:
        _PROG = _build_program()
    return _PROG


def _to_bf(a):
    return np.ascontiguousarray(a.astype(ml_dtypes.bfloat16))


def kernel(hidden_states, ln1_g, ln1_b, ln2_g, ln2_b, wq, bq, wkv, bkv,
           wo, bo, wfc, bfc, wproj, bproj):
    hs = np.asarray(hidden_states, np.float32)
    ln1_g = np.asarray(ln1_g, np.float32)
    ln1_b = np.asarray(ln1_b, np.float32)
    ln2_g = np.asarray(ln2_g, np.float32)
    ln2_b = np.asarray(ln2_b, np.float32)
    wq = np.asarray(wq, np.float32)
    wkv = np.asarray(wkv, np.float32)
    wo = np.asarray(wo, np.float32)
    wfc = np.asarray(wfc, np.float32)
    wproj = np.asarray(wproj, np.float32)

    # Fold LN gains into the following matmuls; fold qk scale into K.
    wq_e = ln1_g[:, None] * wq
    bq_e = np.asarray(bq, np.float32) + ln1_b @ wq
    wkv_e = ln1_g[:, None] * wkv
    bkv_e = np.asarray(bkv, np.float32) + ln1_b @ wkv
    scale = 1.0 / np.sqrt(D)
    wk_e = wkv_e[:, :D] * scale
    bk_e = bkv_e[:D] * scale
    wv_e = wkv_e[:, D:]
    bv_e = bkv_e[D:]
    wfc_e = ln2_g[:, None] * wfc
    bfc_e = np.asarray(bfc, np.float32) + ln2_b @ wfc

    # Host-packed weight layouts: [out-block, partition, k-tile, n] so each
    # band DMA is contiguous per partition line.
    wq_l = _to_bf(wq_e.reshape(HT, 128, NH, 128).transpose(2, 1, 0, 3))
    wk_l = _to_bf(wk_e.reshape(HT, 128, 128).transpose(1, 0, 2))
    wv_l = _to_bf(wv_e.reshape(HT, 128, 128).transpose(1, 0, 2))
    wo_l = _to_bf(wo.reshape(HT, 128, HT, 128).transpose(2, 1, 0, 3))
    wfc_l = _to_bf(wfc_e.reshape(HT, 128, IT, 128).transpose(2, 1, 0, 3))
    wproj_l = _to_bf(wproj.reshape(IT, 128, HT, 128).transpose(2, 1, 0, 3))

    # Negated column sums for the K=1 LN-mean-fold correction matmuls.
    csq_r = _to_bf(-wq_e.sum(axis=0).reshape(1, NH, 128))
    csk_r = _to_bf(-wk_e.sum(axis=0)[None, :])
    csv_r = _to_bf(-wv_e.sum(axis=0)[None, :])

    bq_r = np.ascontiguousarray(bq_e.reshape(NH, 128).T)
    # bk is dropped on-device (a per-(head,q) constant in the logits is
    # softmax-invariant); bv folds into bo exactly (probs sum to 1).
    bo_e = np.asarray(bo, np.float32) + np.tile(bv_e, NH) @ wo
    bo_r = np.ascontiguousarray(bo_e.reshape(HT, 128).T)
    bfc_r = np.ascontiguousarray(bfc_e.reshape(IT, 128).T)
    bproj_r = np.ascontiguousarray(
        np.asarray(bproj, np.float32).reshape(HT, 128).T)

    # Causal masks for the two parity-dependent diagonal k-tiles.
    tri = np.where(np.arange(128)[None, :] >= np.arange(128)[:, None],
                   0.0, NEG).astype(np.float32)          # [k,q]
    tri4 = np.tile(tri, (1, 4))                          # [128, 512] (4 heads)
    zeros4 = np.zeros((128, 512), np.float32)
    neg4 = np.full((128, 512), NEG, np.float32)
    mask_h = [(_to_bf(tri4), _to_bf(neg4)),              # parity 0: (A, B)
              (_to_bf(zeros4), _to_bf(tri4))]            # parity 1: (A, B)

    in_maps = []
    gmaps = []
    for c in range(8):
        b, h = divmod(c, 2)
        gmap = [2 * j + h for j in range(8)]
        gmaps.append(gmap)
        xb = hs[b]                                        # [2048, 2048]
        xt_h = _to_bf(xb.reshape(NCH, 512, HT, 128).transpose(0, 3, 2, 1))
        xqb = xb.reshape(16, 128, H)[gmap].reshape(NQ, H)  # [1024, 2048]
        xq_h = _to_bf(xqb.reshape(2, 512, HT, 128).transpose(0, 3, 2, 1))
        xtq_h = _to_bf(xqb.reshape(2, 512, HT, 128).transpose(2, 0, 3, 1))
        mA, mB = mask_h[h]
        in_maps.append(dict(
            xt=xt_h, xq=xq_h, xtq=xtq_h,
            wq=wq_l, wk=wk_l, wv=wv_l, wo=wo_l, wfc=wfc_l, wproj=wproj_l,
            bq=bq_r, bo=bo_r, bfc=bfc_r, bproj=bproj_r,
            maskA=mA, maskB=mB, csq=csq_r, csk=csk_r, csv=csv_r,
        ))

    res = run_bass_kernel_spmd(_get_prog(), in_maps, core_ids=list(range(8)))
    kernel.last_result = res

    out = np.empty((B, S, H), np.float32)
    for c in range(8):
        b, h = divmod(c, 2)
        resT = np.asarray(res.results[c]["out"]).astype(np.float32)
        blocks = resT.T.reshape(8, 128, H)                # local q-blocks
        for j, g in enumerate(gmaps[c]):
            out[b, g * 128:(g + 1) * 128, :] = blocks[j]
    return out


kernel.last_result = None



# revision 31
# speedup vs baseline: 1.0527x; 1.0072x over previous
"""GPTBigCode transformer block (MQA) on 8 trn2 NeuronCores — v2.

Sharding: data-parallel over batch (4) x parity-interleaved q-block split
(2) per batch element. Core c handles batch c//2 and q-blocks {2j + c%2}.
No collectives; K/V (single MQA head) recomputed per core.

v2 keeps ALL activations feature-on-partition ("T layout") end-to-end —
zero PE transposes. LayerNorm statistics are computed with ones-vector
matmul chains (partition-axis reduction on the tensor engine), per-token
scalars are broadcast back across partitions with K=1 matmuls. Attention
computes transposed scores (keys-on-partition) so softmax-denominators
come from ones-matmuls and probs feed attn@V directly. The softmax
normalization is applied as a per-column multiply on the attention
output. Causal masking of the parity-dependent diagonal zone uses two
per-core mask inputs so the compiled program is identical on all cores.

Weights are host-packed so every weight DMA is contiguous per partition
line; activations never round-trip through DRAM. Matmul inputs bf16;
accumulation, softmax and residual math f32 (residual stream bf16).
"""

import numpy as np
import ml_dtypes

# ---------------------------------------------------------------------------
# Workaround: this container's walrus build rejects >1 sync-wait on
# CTRL-class (Drain) instructions. Split the Tile tail-drain's waits into
# individual wait-carrying NOPs on the SP engine.
import bass_rust
from concourse.tile import TileContext
from concourse.vector_clock import ScopedClock


def _patched_drain_and_barrier(self, tick_clock, wait_clock):
    nc = self.nc
    drain_inst = nc.sync.drain()
    wait_clock.add_sem_waits(
        drain_inst.ins, ScopedClock({None: tick_clock.global_clock})
    )
    si = drain_inst.ins.sync_info
    waits = list(si.on_wait) if si and si.on_wait else []
    if len(waits) > 1:
        drain_inst.ins.sync_info = bass_rust.SyncInfo(
            on_wait=waits[:1],
            on_update=list(si.on_update) if si.on_update else [],
        )
        for w in waits[1:]:
            n = nc.sync.nop(nofuse=True, hint="split_drain_wait")
            n.ins.sync_info = bass_rust.SyncInfo(on_wait=[w], on_update=[])
    nc.all_engine_barrier()
    assert self.sems is not None
    popped = nc._tile_sem_poison_stack.pop()
    assert popped is self._sem_poison
    nc.clear_and_free_semaphores(list(self.sems.allocated().values()))
    nc.all_engine_barrier()


TileContext._drain_and_barrier = _patched_drain_and_barrier


def _split_excess_waits(nc, max_waits=1):
    """Rewrite every instruction carrying more than `max_waits` sem-waits:
    excess waits move onto same-engine NOPs inserted just before it."""
    all_bbs = [bb for fn in nc.m.functions for bb in fn.blocks]
    for bb in all_bbs:
        insts = list(bb.instructions)
        new_list = []
        changed = False
        for inst in insts:
            si = inst.sync_info
            waits = list(si.on_wait) if si and si.on_wait else []
            if len(waits) > max_waits:
                changed = True
                inst.sync_info = bass_rust.SyncInfo(
                    on_wait=waits[:max_waits],
                    on_update=list(si.on_update) if si.on_update else [],
                )
                for w in waits[max_waits:]:
                    nop_bi = nc.engines[inst.engine].nop(
                        nofuse=True, hint="wsplit"
                    )
                    nop = nop_bi.ins
                    cur = nc.cur_bb.bb
                    cl = list(cur.instructions)
                    assert cl and cl[-1].name == nop.name, "nop not appended last"
                    cur.instructions = cl[:-1]
                    nop.sync_info = bass_rust.SyncInfo(on_wait=[w], on_update=[])
                    new_list.append(nop)
            new_list.append(inst)
        if changed:
            bb.instructions = new_list
# ---------------------------------------------------------------------------

import concourse.bass as bass
import concourse.mybir as mybir
from concourse.bass_utils import run_bass_kernel_spmd
from concourse.masks import make_identity

f32 = mybir.dt.float32
bf16 = mybir.dt.bfloat16
AF = mybir.ActivationFunctionType
ALU = mybir.AluOpType

H = 2048
NH = 16
D = 128
INTER = 8192
S = 2048
B = 4
NQ = 1024          # query tokens per core
HT = H // 128      # 16
IT = INTER // 128  # 64
NCH = S // 512     # 4 full-seq chunks
EPS = 1e-5
NEG = -30000.0
INV_H = 1.0 / H


def _ln_rows(nc, rowp, workp2, psB, sum_ps, sumsq_ps, eps_t):
    """[1,512] psum sums -> (m_sb f32, rstd_sb f32) row tiles."""
    m_sb = rowp.tile([1, 512], f32, tag="rows", bufs=2, name="m_sb")
    nc.scalar.mul(m_sb, sum_ps, INV_H)
    v_sb = rowp.tile([1, 512], f32, tag="rows", bufs=2, name="v_sb")
    nc.scalar.mul(v_sb, sumsq_ps, INV_H)
    m2 = workp2.tile([128, 512], f32, tag="t2k", bufs=2, name="m2")
    nc.vector.tensor_mul(m2[0:1, :], m_sb, m_sb)
    nc.vector.tensor_sub(v_sb, v_sb, m2[0:1, :])
    # rstd = exp(-0.5*ln(var+eps)) — keeps the whole row path on ScalarE
    # (DVE reciprocal on a 1-partition row is ~3.3us serial); both steps
    # in place so the rows tag needs only 2 slots.
    nc.scalar.activation(v_sb, v_sb, AF.Ln, bias=eps_t)
    nc.scalar.activation(v_sb, v_sb, AF.Exp, scale=-0.5)
    return m_sb, v_sb


def _ln_chunk_stats(nc, workp, rowp, psB, psC, xc, ones_col, eps_t):
    """LN stats for one [128,16,512] bf16 chunk (raw x, T layout). sum and
    sumsq live in different psum tags so consecutive chunks' stats chains
    double-buffer instead of serializing on one tag pair. Squares are
    batched two k-tiles per ACT op to halve the serial ACT chain that
    otherwise gates the chunk."""
    sum_ps = psB.tile([1, 512], f32, tag="B", bufs=2, name="sum_ps")
    for kt in range(HT):
        nc.tensor.matmul(sum_ps, ones_col, xc[:, kt, :],
                         start=(kt == 0), stop=(kt == HT - 1))
    sqacc = workp.tile([128, 2, 512], bf16, tag="exac", bufs=2, name="sqa")
    nc.scalar.square(sqacc, xc[:, 0:2, :])
    for kt in range(2, HT, 2):
        sq = workp.tile([128, 2, 512], bf16, tag="t2k", bufs=2, name="sq")
        nc.scalar.square(sq, xc[:, kt:kt + 2, :])
        nc.vector.tensor_add(sqacc, sqacc, sq)
    nc.vector.tensor_add(sqacc[:, 0, :], sqacc[:, 0, :], sqacc[:, 1, :])
    sumsq_ps = psC.tile([1, 512], f32, tag="C", bufs=2, name="sumsq_ps")
    nc.tensor.matmul(sumsq_ps, ones_col, sqacc[:, 0, :], start=True, stop=True)
    return _ln_rows(nc, rowp, workp, psB, sum_ps, sumsq_ps, eps_t)


def _bcast_row_bf16(nc, workp, psC, ones_rowf, row_sb, name):
    """[1,512] f32 row -> [128,512] bf16 sbuf broadcast tile."""
    bc_ps = psC.tile([128, 512], f32, tag="C", bufs=2, name=f"{name}_ps")
    nc.tensor.matmul(bc_ps, ones_rowf, row_sb, start=True, stop=True)
    bc_sb = workp.tile([128, 512], bf16, tag="bc1k", bufs=2, name=f"{name}_sb")
    nc.scalar.copy(bc_sb, bc_ps)
    return bc_sb


# LN1 is folded into the projections: out = rstd ∘ (W^T x_raw − m·csW) + b.
# x stays RAW in SBUF (no in-place scale); the mean term is a K=1 rank-1
# correction matmul with -colsum(W) against the plain mean row, and the rstd
# scale is a single per-output-tile DVE multiply with the broadcast tile.
# The K bias is dropped entirely (softmax-invariant) and the V bias is
# folded into bo host-side (attention probs sum to 1).


def _build_program():
    nc = bass.Bass(trn_type="TRN2")

    xt_d = nc.dram_tensor("xt", [NCH, 128, HT, 512], bf16, kind="ExternalInput")
    xq_d = nc.dram_tensor("xq", [2, 128, HT, 512], bf16, kind="ExternalInput")
    xtq_d = nc.dram_tensor("xtq", [HT, 2, 128, 512], bf16, kind="ExternalInput")
    wq_d = nc.dram_tensor("wq", [NH, 128, HT, 128], bf16, kind="ExternalInput")
    wk_d = nc.dram_tensor("wk", [128, HT, 128], bf16, kind="ExternalInput")
    wv_d = nc.dram_tensor("wv", [128, HT, 128], bf16, kind="ExternalInput")
    wo_d = nc.dram_tensor("wo", [HT, 128, HT, 128], bf16, kind="ExternalInput")
    wfc_d = nc.dram_tensor("wfc", [IT, 128, HT, 128], bf16, kind="ExternalInput")
    wproj_d = nc.dram_tensor("wproj", [HT, 128, IT, 128], bf16, kind="ExternalInput")
    bq_d = nc.dram_tensor("bq", [128, NH], f32, kind="ExternalInput")
    bo_d = nc.dram_tensor("bo", [128, HT], f32, kind="ExternalInput")
    bfc_d = nc.dram_tensor("bfc", [128, IT], f32, kind="ExternalInput")
    bproj_d = nc.dram_tensor("bproj", [128, HT], f32, kind="ExternalInput")
    maskA_d = nc.dram_tensor("maskA", [128, 512], bf16, kind="ExternalInput")
    maskB_d = nc.dram_tensor("maskB", [128, 512], bf16, kind="ExternalInput")
    csq_d = nc.dram_tensor("csq", [1, NH, 128], bf16, kind="ExternalInput")
    csk_d = nc.dram_tensor("csk", [1, 128], bf16, kind="ExternalInput")
    csv_d = nc.dram_tensor("csv", [1, 128], bf16, kind="ExternalInput")
    out_d = nc.dram_tensor("out", [H, NQ], bf16, kind="ExternalOutput")

    with TileContext(nc) as tc:
        with (
            tc.tile_pool(name="const", bufs=1) as constp,
            tc.tile_pool(name="big", bufs=1) as bigp,
            tc.tile_pool(name="s32", bufs=2) as s32p,
            tc.tile_pool(name="str16", bufs=2) as strp,
            tc.tile_pool(name="band", bufs=3) as bandp,
            tc.tile_pool(name="work", bufs=2) as workp,
            tc.tile_pool(name="rows", bufs=4) as rowp,
            tc.tile_pool(name="psA", bufs=2, space="PSUM") as psA,
            tc.tile_pool(name="psB", bufs=2, space="PSUM") as psB,
            tc.tile_pool(name="psC", bufs=2, space="PSUM") as psC,
        ):
            # All four full-seq chunks prefetch into one big tile that
            # tag-shares with gT (xc_all is dead before the MLP writes gT,
            # so both fit in one 64KB/partition slot). Chunk 0 is split in
            # two DMAs so compute can start on its first half early.
            xc_all = bigp.tile([128, NCH, HT, 512], bf16, tag="big",
                               name="xc_all")
            nc.sync.dma_start(xc_all[:, 0, 0:8, :], xt_d[0][:, 0:8, :])
            nc.sync.dma_start(xc_all[:, 0, 8:16, :], xt_d[0][:, 8:16, :])
            for ci in range(1, NCH):
                nc.sync.dma_start(xc_all[:, ci], xt_d[ci])

            # ---- constants ----
            ones_col = constp.tile([128, 1], bf16, name="ones_col")
            nc.vector.memset(ones_col, 1.0)
            ones_rowf = constp.tile([1, 128], f32, name="ones_rowf")
            nc.vector.memset(ones_rowf, 1.0)
            eps_t = constp.tile([1, 1], f32, name="eps_t")
            nc.vector.memset(eps_t, EPS)
            bq_sb = constp.tile([128, NH], f32, name="bq_sb")
            nc.sync.dma_start(bq_sb, bq_d[:, :])
            bo_sb = constp.tile([128, HT], f32, name="bo_sb")
            nc.sync.dma_start(bo_sb, bo_d[:, :])
            bfc_sb = constp.tile([128, IT], f32, name="bfc_sb")
            nc.sync.dma_start(bfc_sb, bfc_d[:, :])
            bproj_sb = constp.tile([128, HT], f32, name="bproj_sb")
            nc.sync.dma_start(bproj_sb, bproj_d[:, :])
            maskA = constp.tile([128, 512], bf16, name="maskA")
            nc.sync.dma_start(maskA, maskA_d[:, :])
            maskB = constp.tile([128, 512], bf16, name="maskB")
            nc.sync.dma_start(maskB, maskB_d[:, :])
            csq_sb = constp.tile([1, NH, 128], bf16, name="csq_sb")
            nc.sync.dma_start(csq_sb, csq_d[:, :, :])
            csk_sb = constp.tile([1, 128], bf16, name="csk_sb")
            nc.sync.dma_start(csk_sb, csk_d[:, :])
            csv_sb = constp.tile([1, 128], bf16, name="csv_sb")
            nc.sync.dma_start(csv_sb, csv_d[:, :])
            id_bf = constp.tile([128, 128], bf16, name="id_bf")
            make_identity(nc, id_bf)
            kT_sb = constp.tile([128, S], bf16, name="kT_sb")
            vtm = constp.tile([128, HT, 128], bf16, name="vtm")
            # K/V weights are tiny (512KB each) — load once into two band
            # slots; they stay live across all four chunks of phase A.
            wk_sb = bandp.tile([128, HT, 128], bf16, tag="band4", bufs=3,
                               name="wk_sb")
            nc.sync.dma_start(wk_sb, wk_d[:, :, :])
            wv_sb = bandp.tile([128, HT, 128], bf16, tag="band4", bufs=3,
                               name="wv_sb")
            nc.sync.dma_start(wv_sb, wv_d[:, :, :])
            # Own-q chunks prefetch early too.
            xqcs = []
            for ch in range(2):
                xqc = strp.tile([128, HT, 512], bf16, tag="str16", name="xqc")
                nc.sync.dma_start(xqc, xq_d[ch])
                xqcs.append(xqc)

            # ---- phase A/B: LN1 stats + K/V/Q projections ----
            # Stats for chunk ci+1 are emitted before chunk ci's projections
            # so the serial square/add chain of the next chunk hides under
            # the current chunk's PE-dense projection work.

            def _stats(xc, name):
                m_sb, rstd = _ln_chunk_stats(nc, workp, rowp, psB, psC, xc,
                                             ones_col, eps_t)
                bcr = _bcast_row_bf16(nc, workp, psC, ones_rowf, rstd, name)
                mrow = workp.tile([1, 512], bf16, tag="xq1k", bufs=2,
                                  name="mrow")
                nc.vector.tensor_copy(mrow, m_sb)
                return bcr, mrow

            def _kv_mains(ci):
                """K/V main accumulations on RAW x — no stats dependency, so
                the PE never waits for the square/add chains. The groups stay
                open until the rank-1 mean correction in the epilogue."""
                xc = xc_all[:, ci]
                kps = psA.tile([128, 2, 512], f32, tag="A", bufs=2, name="kps")
                for kt in range(HT):
                    nc.tensor.matmul(kps[:, 0, :], wk_sb[:, kt, :],
                                     xc[:, kt, :], start=(kt == 0), stop=False)
                vps = psA.tile([128, 2, 512], f32, tag="A", bufs=2, name="vps")
                for kt in range(HT):
                    nc.tensor.matmul(vps[:, 0, :], wv_sb[:, kt, :],
                                     xc[:, kt, :], start=(kt == 0), stop=False)
                return kps, vps

            def _kv_epi(ci, kps, vps, bcr, mrow):
                # K^T chunk: [dk=128, 512 tokens]; K bias dropped (softmax-
                # invariant), rstd applied on the output tile.
                nc.tensor.matmul(kps[:, 0, :], csk_sb, mrow,
                                 start=False, stop=True)
                nc.vector.tensor_mul(kT_sb[:, ci * 512:(ci + 1) * 512],
                                     kps[:, 0, :], bcr)
                # V feature-major [dv, 512], then four PE transposes into the
                # token-major vtm the AV matmuls need. V bias folded into bo.
                nc.tensor.matmul(vps[:, 0, :], csv_sb, mrow,
                                 start=False, stop=True)
                vfm = workp.tile([128, 512], bf16, tag="vfm", bufs=2,
                                 name="vfm")
                nc.vector.tensor_mul(vfm, vps[:, 0, :], bcr)
                for tb in range(4):
                    tp = psB.tile([128, 128], bf16, tag="B", bufs=2, name="tp")
                    nc.tensor.transpose(tp, vfm[:, tb * 128:(tb + 1) * 128],
                                        id_bf)
                    nc.vector.tensor_copy(vtm[:, ci * 4 + tb, :], tp)

            def _q_half(ch, bcr, mrow):
                xqc = xqcs[ch]
                for m in range(NH):
                    hg, hh = divmod(m, 4)
                    band = bandp.tile([128, HT, 128], bf16, tag="band4",
                                      bufs=3, name="band")
                    nc.sync.dma_start(band, wq_d[m])
                    qps = psA.tile([128, 2, 512], f32, tag="A", bufs=2,
                                   name="qps")
                    for kt in range(HT):
                        nc.tensor.matmul(qps[:, 0, :], band[:, kt, :],
                                         xqc[:, kt, :],
                                         start=(kt == 0), stop=False)
                    nc.tensor.matmul(qps[:, 0, :], csq_sb[:, m, :], mrow,
                                     start=False, stop=True)
                    qsc = workp.tile([128, 512], bf16, tag="vfm", bufs=2,
                                     name="qsc")
                    nc.vector.tensor_mul(qsc, qps[:, 0, :], bcr)
                    nc.scalar.activation(qTs[ch][:, hg, :, hh, :],
                                         qsc, AF.Identity,
                                         bias=bq_sb[:, m:m + 1])

            # qT layout [dq, hg, j, hh, q]: scores rhs [:, hg, j] is a fully
            # contiguous 512-column block (4 heads x 128 q). Split per 512-q
            # half (j<4 / j>=4) so each half's slot frees right after its
            # last ladder.
            qTs = [
                s32p.tile([128, 4, 4, 4, 128], bf16, tag="s32", bufs=4,
                          name="qT0"),
                s32p.tile([128, 4, 4, 4, 128], bf16, tag="s32", bufs=4,
                          name="qT1"),
            ]

            st = [None] * 6   # 4 full chunks + 2 q-halves
            st[0] = _stats(xc_all[:, 0], "bcr0")
            kv0 = _kv_mains(0)
            st[1] = _stats(xc_all[:, 1], "bcr1")
            _kv_epi(0, *kv0, *st[0])
            kv1 = _kv_mains(1)
            st[2] = _stats(xc_all[:, 2], "bcr2")
            _kv_epi(1, *kv1, *st[1])
            kv2 = _kv_mains(2)
            st[3] = _stats(xc_all[:, 3], "bcr3")
            _kv_epi(2, *kv2, *st[2])
            kv3 = _kv_mains(3)
            st[4] = _stats(xqcs[0], "bcrq0")
            _kv_epi(3, *kv3, *st[3])
            st[5] = _stats(xqcs[1], "bcrq1")
            _q_half(0, *st[4])
            _q_half(1, *st[5])

            # ---- phase C: attention (scoresT, padded extent E=2j+2) ----
            # Two q-blocks' ladders are interleaved per head-group so the PE
            # fills the ACT-exp latency of one block with the other block's
            # matmuls. attnT is split per 512-q half so phase D's first half
            # can interleave with the last (ACT-heavy) ladders.
            attnTs = [
                s32p.tile([128, NH, 512], bf16, tag="s32", bufs=4,
                          name="attnT0"),
                s32p.tile([128, NH, 512], bf16, tag="s32", bufs=4,
                          name="attnT1"),
            ]

            def _attn_step(hg, j, p0, kts, exacc, av_ps):
                """One 2-kt step of block j: two scores matmuls into a 2-bank
                psum tile, one exp over both, av accumulation on the PE and
                elementwise exp accumulation on the (otherwise idle) DVE —
                the softmax denominator then needs only ONE ones-matmul per
                block instead of one per k-tile. Causal masking is a 0/1
                multiply on exp(s) (exp(s+M) = exp(s)·exp(M)) — on the DVE,
                keeping the PE free of mask matmuls."""
                E = len(kts)
                sc = psA.tile([128, 2, 512], f32, tag="A", bufs=2, name="sc")
                for dk in range(2):
                    kt = kts[p0 + dk]
                    masked = kt >= E - 2
                    nc.tensor.matmul(
                        sc[:, dk, :], kT_sb[:, kt * 128:(kt + 1) * 128],
                        qTs[j // 4][:, hg, j % 4], start=True, stop=not masked,
                    )
                    if masked:
                        # mask add on the PE: sc += I^T @ mask. Keeping this
                        # inside the accumulation group avoids a cross-engine
                        # hop on the exp->AV critical path (measured: a DVE
                        # 0/1-multiply here cost ~1us/block in AV stalls).
                        nc.tensor.matmul(sc[:, dk, :], id_bf,
                                         maskA if kt == E - 2 else maskB,
                                         start=False, stop=True)
                ex = workp.tile([128, 2, 512], bf16, tag="t2k", bufs=2,
                                name="ex")
                nc.scalar.activation(ex, sc, AF.Exp)
                if p0 == 0:
                    nc.vector.tensor_copy(exacc, ex[:, 0, :])
                else:
                    nc.vector.tensor_add(exacc, exacc, ex[:, 0, :])
                nc.vector.tensor_add(exacc, exacc, ex[:, 1, :])
                for dk in range(2):
                    idx = p0 + dk
                    kt = kts[idx]
                    nc.tensor.matmul(av_ps, vtm[:, kt, :], ex[:, dk, :],
                                     start=(idx == 0), stop=(idx == E - 1))

            def _attn_tail(hg, j, exacc, av_ps):
                den_ps = psB.tile([1, 512], f32, tag="B", bufs=2,
                                  name="den_ps")
                nc.tensor.matmul(den_ps, ones_col, exacc, start=True, stop=True)
                lnd = rowp.tile([1, 512], f32, tag="rows", bufs=2, name="lnd")
                nc.scalar.activation(lnd, den_ps, AF.Ln)
                nc.scalar.activation(lnd, lnd, AF.Exp, scale=-1.0)
                rec = lnd
                bcr_ps = psB.tile([128, 512], f32, tag="B", bufs=2,
                                  name="bcr_ps")
                nc.tensor.matmul(bcr_ps, ones_rowf, rec, start=True, stop=True)
                bcr_sb = workp.tile([128, 512], bf16, tag="bc1k", bufs=2,
                                    name="bcr_sb")
                # DVE copy: ACT is the bottleneck engine in this phase.
                nc.vector.tensor_copy(bcr_sb, bcr_ps)
                jj = j % 4
                nc.vector.tensor_mul(
                    attnTs[j // 4][:, hg * 4:(hg + 1) * 4,
                                   jj * 128:(jj + 1) * 128],
                    av_ps, bcr_sb,
                )

            def _ladder(hg, jp):
                j0, j1 = 2 * jp, 2 * jp + 1
                E0, E1 = 2 * j0 + 2, 2 * j1 + 2
                kts0 = [E0 - 2, E0 - 1] + list(range(E0 - 2))
                kts1 = [E1 - 2, E1 - 1] + list(range(E1 - 2))
                ea0 = workp.tile([128, 512], bf16, tag="exac", bufs=2,
                                 name="ea0")
                av0 = psC.tile([128, 512], f32, tag="C", bufs=2, name="av0")
                ea1 = workp.tile([128, 512], bf16, tag="exac", bufs=2,
                                 name="ea1")
                av1 = psC.tile([128, 512], f32, tag="C", bufs=2, name="av1")
                for p0 in range(0, E1, 2):
                    if p0 < E0:
                        _attn_step(hg, j0, p0, kts0, ea0, av0)
                    elif p0 == E0:
                        _attn_tail(hg, j0, ea0, av0)
                    _attn_step(hg, j1, p0, kts1, ea1, av1)
                _attn_tail(hg, j1, ea1, av1)

            # ---- phase D pieces (emitted interleaved with phase C) ----
            ys = [
                s32p.tile([128, HT, 512], bf16, tag="s32", bufs=4, name="y0"),
                None,   # y1 allocated after qT/attnT0 die
            ]

            def _d_ob(ch, ob):
                band = bandp.tile([128, HT, 128], bf16, tag="band4", bufs=3,
                                  name="band")
                nc.sync.dma_start(band, wo_d[ob])
                wps = psA.tile([128, 2, 512], f32, tag="A", bufs=2,
                               name="wps")
                for ht in range(HT):
                    nc.tensor.matmul(wps[:, 0, :], band[:, ht, :],
                                     attnTs[ch][:, ht, :],
                                     start=(ht == 0), stop=(ht == HT - 1))
                xqt = workp.tile([128, 512], bf16, tag="xq1k", bufs=2,
                                 name="xqt")
                nc.sync.dma_start(xqt, xtq_d[ob, ch])
                nc.vector.scalar_tensor_tensor(
                    out=ys[ch][:, ob, :],
                    in0=wps[:, 0, :], scalar=bo_sb[:, ob:ob + 1],
                    in1=xqt, op0=ALU.add, op1=ALU.add,
                )

            # jp0/jp1 ladders run alone (short extents); the ACT-heavy
            # jp2/jp3 ladders interleave with phase-D ch0 blocks so the PE
            # chews wo matmuls while ACT drains the exp queue.
            for jp in range(2):
                for hg in range(4):
                    _ladder(hg, jp)
            dch0 = iter(range(HT))
            for jp in (2, 3):
                for hg in range(4):
                    _ladder(hg, jp)
                    _d_ob(0, next(dch0))
                    _d_ob(0, next(dch0))

            def _ln2_acc(ch):
                """LN2 elementwise stats accumulation for one chunk — pure
                ACT/DVE work (two k-tiles per op), emitted right after the
                chunk's y completes so it hides under PE-dense stretches."""
                yc = ys[ch]
                yacc = workp.tile([128, 2, 512], bf16, tag="exac", bufs=2,
                                  name="yacc")
                nc.vector.tensor_add(yacc, yc[:, 0:2, :], yc[:, 2:4, :])
                for kt in range(4, HT, 2):
                    nc.vector.tensor_add(yacc, yacc, yc[:, kt:kt + 2, :])
                nc.vector.tensor_add(yacc[:, 0, :], yacc[:, 0, :],
                                     yacc[:, 1, :])
                sqacc = workp.tile([128, 2, 512], bf16, tag="exac", bufs=2,
                                   name="sqacc")
                nc.scalar.square(sqacc, yc[:, 0:2, :])
                for kt in range(2, HT, 2):
                    sq = workp.tile([128, 2, 512], bf16, tag="t2k", bufs=2,
                                    name="sq2")
                    nc.scalar.square(sq, yc[:, kt:kt + 2, :])
                    nc.vector.tensor_add(sqacc, sqacc, sq)
                nc.vector.tensor_add(sqacc[:, 0, :], sqacc[:, 0, :],
                                     sqacc[:, 1, :])
                return yacc, sqacc

            def _ln2_fin(ch, yacc, sqacc):
                """LN2 reduction matmuls + broadcast + normalize."""
                yc = ys[ch]
                sum_ps = psB.tile([1, 512], f32, tag="B", bufs=2, name="l2sum")
                nc.tensor.matmul(sum_ps, ones_col, yacc[:, 0, :], start=True,
                                 stop=True)
                sumsq_ps = psC.tile([1, 512], f32, tag="C", bufs=2, name="l2sq")
                nc.tensor.matmul(sumsq_ps, ones_col, sqacc[:, 0, :],
                                 start=True, stop=True)
                m_sb, rstd = _ln_rows(nc, rowp, workp, psB, sum_ps, sumsq_ps,
                                      eps_t)
                bcm = _bcast_row_bf16(nc, workp, psC, ones_rowf, m_sb, "l2bcm")
                bcr = _bcast_row_bf16(nc, workp, psC, ones_rowf, rstd, "l2bcr")
                ln2s = s32p.tile([128, HT, 512], bf16, tag="s32", bufs=4,
                                 name="ln2s")
                for kt in range(HT):
                    nc.vector.tensor_sub(ln2s[:, kt, :], yc[:, kt, :], bcm)
                    nc.vector.tensor_mul(ln2s[:, kt, :], ln2s[:, kt, :], bcr)
                return ln2s

            acc0 = _ln2_acc(0)
            ys[1] = s32p.tile([128, HT, 512], bf16, tag="s32", bufs=4,
                              name="y1")
            for ob in range(HT):
                _d_ob(1, ob)

            # ---- phase E: LN2 + MLP + residual -> out (per 512-token chunk)
            # Emission order keeps each chunk's stats chain off the critical
            # path: fin0 right after D (acc0 ran under D-ch1), acc1 under
            # fc-ch0, fin1 between fc-ch0 and proj-ch0.
            ln2s_pair = [_ln2_fin(0, *acc0), None]
            acc1 = _ln2_acc(1)
            for ch in range(2):
                cols = slice(ch * 512, (ch + 1) * 512)
                ln2s = ln2s_pair[ch]

                gT = bigp.tile([128, IT, 512], bf16, tag="big", name="gT")
                for mb in range(IT):
                    band = bandp.tile([128, HT, 128], bf16, tag="band4",
                                      bufs=3, name="band")
                    nc.sync.dma_start(band, wfc_d[mb])
                    fps = psA.tile([128, 2, 512], f32, tag="A", bufs=2,
                                   name="fps")
                    for kt in range(HT):
                        nc.tensor.matmul(fps[:, 0, :], band[:, kt, :],
                                         ln2s[:, kt, :],
                                         start=(kt == 0), stop=(kt == HT - 1))
                    nc.scalar.activation(gT[:, mb, :], fps[:, 0, :],
                                         AF.Gelu_apprx_tanh,
                                         bias=bfc_sb[:, mb:mb + 1])
                if ch == 0:
                    ln2s_pair[1] = _ln2_fin(1, *acc1)

                for ob in range(HT):
                    pband = strp.tile([128, IT, 128], bf16, tag="str16",
                                      name="pband")
                    nc.sync.dma_start(pband, wproj_d[ob])
                    pps = psA.tile([128, 2, 512], f32, tag="A", bufs=2,
                                   name="pps")
                    for mt in range(IT):
                        nc.tensor.matmul(pps[:, 0, :], pband[:, mt, :],
                                         gT[:, mt, :],
                                         start=(mt == 0), stop=(mt == IT - 1))
                    osb = workp.tile([128, 512], bf16, tag="f2k", bufs=2,
                                     name="osb")
                    nc.vector.scalar_tensor_tensor(
                        out=osb, in0=pps[:, 0, :],
                        scalar=bproj_sb[:, ob:ob + 1],
                        in1=ys[ch][:, ob, :], op0=ALU.add, op1=ALU.add,
                    )
                    nc.sync.dma_start(
                        out_d[ob * 128:(ob + 1) * 128, cols], osb
                    )
    _split_excess_waits(nc)
    return nc


_PROG = None


def _get_prog():
    global _PROG
    if _PROG is None:
        _PROG = _build_program()
    return _PROG


def _to_bf(a):
    return np.ascontiguousarray(a.astype(ml_dtypes.bfloat16))


def kernel(hidden_states, ln1_g, ln1_b, ln2_g, ln2_b, wq, bq, wkv, bkv,
           wo, bo, wfc, bfc, wproj, bproj):
    hs = np.asarray(hidden_states, np.float32)
    ln1_g = np.asarray(ln1_g, np.float32)
    ln1_b = np.asarray(ln1_b, np.float32)
    ln2_g = np.asarray(ln2_g, np.float32)
    ln2_b = np.asarray(ln2_b, np.float32)
    wq = np.asarray(wq, np.float32)
    wkv = np.asarray(wkv, np.float32)
    wo = np.asarray(wo, np.float32)
    wfc = np.asarray(wfc, np.float32)
    wproj = np.asarray(wproj, np.float32)

    # Fold LN gains into the following matmuls; fold qk scale into K.
    wq_e = ln1_g[:, None] * wq
    bq_e = np.asarray(bq, np.float32) + ln1_b @ wq
    wkv_e = ln1_g[:, None] * wkv
    bkv_e = np.asarray(bkv, np.float32) + ln1_b @ wkv
    scale = 1.0 / np.sqrt(D)
    wk_e = wkv_e[:, :D] * scale
    bk_e = bkv_e[:D] * scale
    wv_e = wkv_e[:, D:]
    bv_e = bkv_e[D:]
    wfc_e = ln2_g[:, None] * wfc
    bfc_e = np.asarray(bfc, np.float32) + ln2_b @ wfc

    # Host-packed weight layouts: [out-block, partition, k-tile, n] so each
    # band DMA is contiguous per partition line.
    wq_l = _to_bf(wq_e.reshape(HT, 128, NH, 128).transpose(2, 1, 0, 3))
    wk_l = _to_bf(wk_e.reshape(HT, 128, 128).transpose(1, 0, 2))
    wv_l = _to_bf(wv_e.reshape(HT, 128, 128).transpose(1, 0, 2))
    wo_l = _to_bf(wo.reshape(HT, 128, HT, 128).transpose(2, 1, 0, 3))
    wfc_l = _to_bf(wfc_e.reshape(HT, 128, IT, 128).transpose(2, 1, 0, 3))
    wproj_l = _to_bf(wproj.reshape(IT, 128, HT, 128).transpose(2, 1, 0, 3))

    # Negated column sums for the K=1 LN-mean-fold correction matmuls.
    csq_r = _to_bf(-wq_e.sum(axis=0).reshape(1, NH, 128))
    csk_r = _to_bf(-wk_e.sum(axis=0)[None, :])
    csv_r = _to_bf(-wv_e.sum(axis=0)[None, :])

    bq_r = np.ascontiguousarray(bq_e.reshape(NH, 128).T)
    # bk is dropped on-device (a per-(head,q) constant in the logits is
    # softmax-invariant); bv folds into bo exactly (probs sum to 1).
    bo_e = np.asarray(bo, np.float32) + np.tile(bv_e, NH) @ wo
    bo_r = np.ascontiguousarray(bo_e.reshape(HT, 128).T)
    bfc_r = np.ascontiguousarray(bfc_e.reshape(IT, 128).T)
    bproj_r = np.ascontiguousarray(
        np.asarray(bproj, np.float32).reshape(HT, 128).T)

    # Causal masks for the two parity-dependent diagonal k-tiles.
    tri = np.where(np.arange(128)[None, :] >= np.arange(128)[:, None],
                   0.0, NEG).astype(np.float32)          # [k,q]
    tri4 = np.tile(tri, (1, 4))                          # [128, 512] (4 heads)
    zeros4 = np.zeros((128, 512), np.float32)
    neg4 = np.full((128, 512), NEG, np.float32)
    mask_h = [(_to_bf(tri4), _to_bf(neg4)),              # parity 0: (A, B)
              (_to_bf(zeros4), _to_bf(tri4))]            # parity 1: (A, B)

    in_maps = []
    gmaps = []
    for c in range(8):
        b, h = divmod(c, 2)
        gmap = [2 * j + h for j in range(8)]
        gmaps.append(gmap)
        xb = hs[b]                                        # [2048, 2048]
        xt_h = _to_bf(xb.reshape(NCH, 512, HT, 128).transpose(0, 3, 2, 1))
        xqb = xb.reshape(16, 128, H)[gmap].reshape(NQ, H)  # [1024, 2048]
        xq_h = _to_bf(xqb.reshape(2, 512, HT, 128).transpose(0, 3, 2, 1))
        xtq_h = _to_bf(xqb.reshape(2, 512, HT, 128).transpose(2, 0, 3, 1))
        mA, mB = mask_h[h]
        in_maps.append(dict(
            xt=xt_h, xq=xq_h, xtq=xtq_h,
            wq=wq_l, wk=wk_l, wv=wv_l, wo=wo_l, wfc=wfc_l, wproj=wproj_l,
            bq=bq_r, bo=bo_r, bfc=bfc_r, bproj=bproj_r,
            maskA=mA, maskB=mB, csq=csq_r, csk=csk_r, csv=csv_r,
        ))

    res = run_bass_kernel_spmd(_get_prog(), in_maps, core_ids=list(range(8)))
    kernel.last_result = res

    out = np.empty((B, S, H), np.float32)
    for c in range(8):
        b, h = divmod(c, 2)
        resT = np.asarray(res.results[c]["out"]).astype(np.float32)
        blocks = resT.T.reshape(8, 128, H)                # local q-blocks
        for j, g in enumerate(gmaps[c]):
            out[b, g * 128:(g + 1) * 128, :] = blocks[j]
    return out


kernel.last_result = None



# revision 40
# speedup vs baseline: 1.0687x; 1.0153x over previous
"""GPTBigCode transformer block (MQA) on 8 trn2 NeuronCores — v2.

Sharding: data-parallel over batch (4) x parity-interleaved q-block split
(2) per batch element. Core c handles batch c//2 and q-blocks {2j + c%2}.
No collectives; K/V (single MQA head) recomputed per core.

v2 keeps ALL activations feature-on-partition ("T layout") end-to-end —
zero PE transposes. LayerNorm statistics are computed with ones-vector
matmul chains (partition-axis reduction on the tensor engine), per-token
scalars are broadcast back across partitions with K=1 matmuls. Attention
computes transposed scores (keys-on-partition) so softmax-denominators
come from ones-matmuls and probs feed attn@V directly. The softmax
normalization is applied as a per-column multiply on the attention
output. Causal masking of the parity-dependent diagonal zone uses two
per-core mask inputs so the compiled program is identical on all cores.

Weights are host-packed so every weight DMA is contiguous per partition
line; activations never round-trip through DRAM. Matmul inputs bf16;
accumulation, softmax and residual math f32 (residual stream bf16).
"""

import numpy as np
import ml_dtypes

# ---------------------------------------------------------------------------
# Workaround: this container's walrus build rejects >1 sync-wait on
# CTRL-class (Drain) instructions. Split the Tile tail-drain's waits into
# individual wait-carrying NOPs on the SP engine.
import bass_rust
from concourse.tile import TileContext
from concourse.vector_clock import ScopedClock


def _patched_drain_and_barrier(self, tick_clock, wait_clock):
    nc = self.nc
    drain_inst = nc.sync.drain()
    wait_clock.add_sem_waits(
        drain_inst.ins, ScopedClock({None: tick_clock.global_clock})
    )
    si = drain_inst.ins.sync_info
    waits = list(si.on_wait) if si and si.on_wait else []
    if len(waits) > 1:
        drain_inst.ins.sync_info = bass_rust.SyncInfo(
            on_wait=waits[:1],
            on_update=list(si.on_update) if si.on_update else [],
        )
        for w in waits[1:]:
            n = nc.sync.nop(nofuse=True, hint="split_drain_wait")
            n.ins.sync_info = bass_rust.SyncInfo(on_wait=[w], on_update=[])
    nc.all_engine_barrier()
    assert self.sems is not None
    popped = nc._tile_sem_poison_stack.pop()
    assert popped is self._sem_poison
    nc.clear_and_free_semaphores(list(self.sems.allocated().values()))
    nc.all_engine_barrier()


TileContext._drain_and_barrier = _patched_drain_and_barrier


def _split_excess_waits(nc, max_waits=1):
    """Rewrite every instruction carrying more than `max_waits` sem-waits:
    excess waits move onto same-engine NOPs inserted just before it."""
    all_bbs = [bb for fn in nc.m.functions for bb in fn.blocks]
    for bb in all_bbs:
        insts = list(bb.instructions)
        new_list = []
        changed = False
        for inst in insts:
            si = inst.sync_info
            waits = list(si.on_wait) if si and si.on_wait else []
            if len(waits) > max_waits:
                changed = True
                inst.sync_info = bass_rust.SyncInfo(
                    on_wait=waits[:max_waits],
                    on_update=list(si.on_update) if si.on_update else [],
                )
                for w in waits[max_waits:]:
                    nop_bi = nc.engines[inst.engine].nop(
                        nofuse=True, hint="wsplit"
                    )
                    nop = nop_bi.ins
                    cur = nc.cur_bb.bb
                    cl = list(cur.instructions)
                    assert cl and cl[-1].name == nop.name, "nop not appended last"
                    cur.instructions = cl[:-1]
                    nop.sync_info = bass_rust.SyncInfo(on_wait=[w], on_update=[])
                    new_list.append(nop)
            new_list.append(inst)
        if changed:
            bb.instructions = new_list
# ---------------------------------------------------------------------------

import concourse.bass as bass
import concourse.mybir as mybir
from concourse.bass_utils import run_bass_kernel_spmd
from concourse.masks import make_identity

f32 = mybir.dt.float32
bf16 = mybir.dt.bfloat16
AF = mybir.ActivationFunctionType
ALU = mybir.AluOpType

H = 2048
NH = 16
D = 128
INTER = 8192
S = 2048
B = 4
NQ = 1024          # query tokens per core
HT = H // 128      # 16
IT = INTER // 128  # 64
NCH = S // 512     # 4 full-seq chunks
EPS = 1e-5
NEG = -30000.0
INV_H = 1.0 / H


def _ln_rows(nc, rowp, workp2, psB, sum_ps, sumsq_ps, eps_t):
    """[1,512] psum sums -> (m_sb f32, rstd_sb f32) row tiles."""
    m_sb = rowp.tile([1, 512], f32, tag="rows", bufs=2, name="m_sb")
    nc.scalar.mul(m_sb, sum_ps, INV_H)
    v_sb = rowp.tile([1, 512], f32, tag="rows", bufs=2, name="v_sb")
    nc.scalar.mul(v_sb, sumsq_ps, INV_H)
    m2 = workp2.tile([128, 512], f32, tag="t2k", bufs=2, name="m2")
    nc.vector.tensor_mul(m2[0:1, :], m_sb, m_sb)
    nc.vector.tensor_sub(v_sb, v_sb, m2[0:1, :])
    # rstd = exp(-0.5*ln(var+eps)) — keeps the whole row path on ScalarE
    # (DVE reciprocal on a 1-partition row is ~3.3us serial); both steps
    # in place so the rows tag needs only 2 slots.
    nc.scalar.activation(v_sb, v_sb, AF.Ln, bias=eps_t)
    nc.scalar.activation(v_sb, v_sb, AF.Exp, scale=-0.5)
    return m_sb, v_sb


def _ln_chunk_stats(nc, workp, rowp, psB, psC, xc, ones_col, eps_t):
    """LN stats for one [128,16,512] bf16 chunk (raw x, T layout). sum and
    sumsq live in different psum tags so consecutive chunks' stats chains
    double-buffer instead of serializing on one tag pair. Squares are
    batched two k-tiles per ACT op to halve the serial ACT chain that
    otherwise gates the chunk."""
    sum_ps = psB.tile([1, 512], f32, tag="B", bufs=2, name="sum_ps")
    for kt in range(HT):
        nc.tensor.matmul(sum_ps, ones_col, xc[:, kt, :],
                         start=(kt == 0), stop=(kt == HT - 1))
    sqacc = workp.tile([128, 2, 512], bf16, tag="exac", bufs=2, name="sqa")
    nc.scalar.square(sqacc, xc[:, 0:2, :])
    for kt in range(2, HT, 2):
        sq = workp.tile([128, 2, 512], bf16, tag="t2k", bufs=2, name="sq")
        nc.scalar.square(sq, xc[:, kt:kt + 2, :])
        nc.vector.tensor_add(sqacc, sqacc, sq)
    nc.vector.tensor_add(sqacc[:, 0, :], sqacc[:, 0, :], sqacc[:, 1, :])
    sumsq_ps = psC.tile([1, 512], f32, tag="C", bufs=2, name="sumsq_ps")
    nc.tensor.matmul(sumsq_ps, ones_col, sqacc[:, 0, :], start=True, stop=True)
    return _ln_rows(nc, rowp, workp, psB, sum_ps, sumsq_ps, eps_t)


def _bcast_row_bf16(nc, workp, psC, ones_rowf, row_sb, name):
    """[1,512] f32 row -> [128,512] bf16 sbuf broadcast tile."""
    bc_ps = psC.tile([128, 512], f32, tag="C", bufs=2, name=f"{name}_ps")
    nc.tensor.matmul(bc_ps, ones_rowf, row_sb, start=True, stop=True)
    bc_sb = workp.tile([128, 512], bf16, tag="bc1k", bufs=2, name=f"{name}_sb")
    nc.scalar.copy(bc_sb, bc_ps)
    return bc_sb


# LN1 is folded into the projections: out = rstd ∘ (W^T x_raw − m·csW) + b.
# x stays RAW in SBUF (no in-place scale); the mean term is a K=1 rank-1
# correction matmul with -colsum(W) against the plain mean row, and the rstd
# scale is a single per-output-tile DVE multiply with the broadcast tile.
# The K bias is dropped entirely (softmax-invariant) and the V bias is
# folded into bo host-side (attention probs sum to 1).


def _build_program():
    nc = bass.Bass(trn_type="TRN2")

    xt_d = nc.dram_tensor("xt", [NCH, 128, HT, 512], bf16, kind="ExternalInput")
    xq_d = nc.dram_tensor("xq", [2, 128, HT, 512], bf16, kind="ExternalInput")
    xtq_d = nc.dram_tensor("xtq", [HT, 2, 128, 512], bf16, kind="ExternalInput")
    wq_d = nc.dram_tensor("wq", [NH, 128, HT, 128], bf16, kind="ExternalInput")
    wk_d = nc.dram_tensor("wk", [128, HT, 128], bf16, kind="ExternalInput")
    wv_d = nc.dram_tensor("wv", [128, HT, 128], bf16, kind="ExternalInput")
    wo_d = nc.dram_tensor("wo", [HT, 128, HT, 128], bf16, kind="ExternalInput")
    wfc_d = nc.dram_tensor("wfc", [IT, 128, HT, 128], bf16, kind="ExternalInput")
    wproj_d = nc.dram_tensor("wproj", [HT, 128, IT, 128], bf16, kind="ExternalInput")
    bq_d = nc.dram_tensor("bq", [128, NH], f32, kind="ExternalInput")
    bo_d = nc.dram_tensor("bo", [128, HT], f32, kind="ExternalInput")
    bfc_d = nc.dram_tensor("bfc", [128, IT], f32, kind="ExternalInput")
    bproj_d = nc.dram_tensor("bproj", [128, HT], f32, kind="ExternalInput")
    maskA_d = nc.dram_tensor("maskA", [128, 512], bf16, kind="ExternalInput")
    maskB_d = nc.dram_tensor("maskB", [128, 512], bf16, kind="ExternalInput")
    csq_d = nc.dram_tensor("csq", [1, NH, 128], bf16, kind="ExternalInput")
    csk_d = nc.dram_tensor("csk", [1, 128], bf16, kind="ExternalInput")
    csv_d = nc.dram_tensor("csv", [1, 128], bf16, kind="ExternalInput")
    out_d = nc.dram_tensor("out", [H, NQ], bf16, kind="ExternalOutput")

    with TileContext(nc) as tc:
        with (
            tc.tile_pool(name="const", bufs=1) as constp,
            tc.tile_pool(name="big", bufs=1) as bigp,
            tc.tile_pool(name="s32", bufs=2) as s32p,
            tc.tile_pool(name="str16", bufs=2) as strp,
            tc.tile_pool(name="band", bufs=3) as bandp,
            tc.tile_pool(name="work", bufs=2) as workp,
            tc.tile_pool(name="rows", bufs=4) as rowp,
            tc.tile_pool(name="psA", bufs=2, space="PSUM") as psA,
            tc.tile_pool(name="psB", bufs=2, space="PSUM") as psB,
            tc.tile_pool(name="psC", bufs=2, space="PSUM") as psC,
        ):
            # All four full-seq chunks prefetch into one big tile that
            # tag-shares with gT (xc_all is dead before the MLP writes gT,
            # so both fit in one 64KB/partition slot). Chunk 0 is split in
            # two DMAs so compute can start on its first half early; the
            # later chunks' DMAs are issued after chunk 0 + the K/V weights
            # so they don't steal packet-round-robin bandwidth from the
            # transfers that gate the first matmuls.
            xc_all = bigp.tile([128, NCH, HT, 512], bf16, tag="big",
                               name="xc_all")
            nc.sync.dma_start(xc_all[:, 0, 0:8, :], xt_d[0][:, 0:8, :])
            nc.sync.dma_start(xc_all[:, 0, 8:16, :], xt_d[0][:, 8:16, :])

            # ---- constants ----
            ones_col = constp.tile([128, 1], bf16, name="ones_col")
            nc.vector.memset(ones_col, 1.0)
            ones_rowf = constp.tile([1, 128], f32, name="ones_rowf")
            nc.vector.memset(ones_rowf, 1.0)
            eps_t = constp.tile([1, 1], f32, name="eps_t")
            nc.vector.memset(eps_t, EPS)
            bq_sb = constp.tile([128, NH], f32, name="bq_sb")
            nc.sync.dma_start(bq_sb, bq_d[:, :])
            bo_sb = constp.tile([128, HT], f32, name="bo_sb")
            nc.sync.dma_start(bo_sb, bo_d[:, :])
            bfc_sb = constp.tile([128, IT], f32, name="bfc_sb")
            nc.sync.dma_start(bfc_sb, bfc_d[:, :])
            bproj_sb = constp.tile([128, HT], f32, name="bproj_sb")
            nc.sync.dma_start(bproj_sb, bproj_d[:, :])
            maskA = constp.tile([128, 512], bf16, name="maskA")
            nc.sync.dma_start(maskA, maskA_d[:, :])
            maskB = constp.tile([128, 512], bf16, name="maskB")
            nc.sync.dma_start(maskB, maskB_d[:, :])
            csq_sb = constp.tile([1, NH, 128], bf16, name="csq_sb")
            nc.sync.dma_start(csq_sb, csq_d[:, :, :])
            csk_sb = constp.tile([1, 128], bf16, name="csk_sb")
            nc.sync.dma_start(csk_sb, csk_d[:, :])
            csv_sb = constp.tile([1, 128], bf16, name="csv_sb")
            nc.sync.dma_start(csv_sb, csv_d[:, :])
            id_bf = constp.tile([128, 128], bf16, name="id_bf")
            make_identity(nc, id_bf)
            kT_sb = constp.tile([128, S], bf16, name="kT_sb")
            vtm = constp.tile([128, HT, 128], bf16, name="vtm")
            # K/V weights are tiny (512KB each) — load once into two band
            # slots; they stay live across all four chunks of phase A.
            wk_sb = bandp.tile([128, HT, 128], bf16, tag="band4", bufs=3,
                               name="wk_sb")
            nc.sync.dma_start(wk_sb, wk_d[:, :, :])
            wv_sb = bandp.tile([128, HT, 128], bf16, tag="band4", bufs=3,
                               name="wv_sb")
            nc.sync.dma_start(wv_sb, wv_d[:, :, :])
            # Remaining full-seq chunks + own-q chunks prefetch next.
            nc.sync.dma_start(xc_all[:, 1], xt_d[1])
            xqcs = []
            for ch in range(2):
                xqc = strp.tile([128, HT, 512], bf16, tag="str16", name="xqc")
                xqcs.append(xqc)
            nc.sync.dma_start(xqcs[0], xq_d[0])
            nc.sync.dma_start(xc_all[:, 2], xt_d[2])
            nc.sync.dma_start(xc_all[:, 3], xt_d[3])
            nc.sync.dma_start(xqcs[1], xq_d[1])

            # ---- phase A/B: LN1 stats + K/V/Q projections ----
            # Stats for chunk ci+1 are emitted before chunk ci's projections
            # so the serial square/add chain of the next chunk hides under
            # the current chunk's PE-dense projection work.

            def _stats(xc, name):
                m_sb, rstd = _ln_chunk_stats(nc, workp, rowp, psB, psC, xc,
                                             ones_col, eps_t)
                bcr = _bcast_row_bf16(nc, workp, psC, ones_rowf, rstd, name)
                mrow = workp.tile([1, 512], bf16, tag="xq1k", bufs=2,
                                  name="mrow")
                nc.vector.tensor_copy(mrow, m_sb)
                return bcr, mrow

            def _kv_mains(ci):
                """K/V main accumulations on RAW x — no stats dependency, so
                the PE never waits for the square/add chains. The groups stay
                open until the rank-1 mean correction in the epilogue."""
                xc = xc_all[:, ci]
                kps = psA.tile([128, 2, 512], f32, tag="A", bufs=2, name="kps")
                for kt in range(HT):
                    nc.tensor.matmul(kps[:, 0, :], wk_sb[:, kt, :],
                                     xc[:, kt, :], start=(kt == 0), stop=False)
                vps = psA.tile([128, 2, 512], f32, tag="A", bufs=2, name="vps")
                for kt in range(HT):
                    nc.tensor.matmul(vps[:, 0, :], wv_sb[:, kt, :],
                                     xc[:, kt, :], start=(kt == 0), stop=False)
                return kps, vps

            def _kv_epi(ci, kps, vps, bcr, mrow):
                # K^T chunk: [dk=128, 512 tokens]; K bias dropped (softmax-
                # invariant), rstd applied on the output tile.
                nc.tensor.matmul(kps[:, 0, :], csk_sb, mrow,
                                 start=False, stop=True)
                nc.vector.tensor_mul(kT_sb[:, ci * 512:(ci + 1) * 512],
                                     kps[:, 0, :], bcr)
                # V feature-major [dv, 512], then four PE transposes into the
                # token-major vtm the AV matmuls need. V bias folded into bo.
                nc.tensor.matmul(vps[:, 0, :], csv_sb, mrow,
                                 start=False, stop=True)
                vfm = workp.tile([128, 512], bf16, tag="vfm", bufs=2,
                                 name="vfm")
                nc.vector.tensor_mul(vfm, vps[:, 0, :], bcr)
                for tb in range(4):
                    tp = psB.tile([128, 128], bf16, tag="B", bufs=2, name="tp")
                    nc.tensor.transpose(tp, vfm[:, tb * 128:(tb + 1) * 128],
                                        id_bf)
                    nc.vector.tensor_copy(vtm[:, ci * 4 + tb, :], tp)

            def _q_head(ch, bcr, mrow, m):
                xqc = xqcs[ch]
                hg, hh = divmod(m, 4)
                band = bandp.tile([128, HT, 128], bf16, tag="band4",
                                  bufs=3, name="band")
                nc.sync.dma_start(band, wq_d[m])
                qps = psA.tile([128, 2, 512], f32, tag="A", bufs=2,
                               name="qps")
                for kt in range(HT):
                    nc.tensor.matmul(qps[:, 0, :], band[:, kt, :],
                                     xqc[:, kt, :],
                                     start=(kt == 0), stop=False)
                nc.tensor.matmul(qps[:, 0, :], csq_sb[:, m, :], mrow,
                                 start=False, stop=True)
                qsc = workp.tile([128, 512], bf16, tag="vfm", bufs=2,
                                 name="qsc")
                nc.vector.tensor_mul(qsc, qps[:, 0, :], bcr)
                nc.scalar.activation(qTs[ch][:, hg, :, hh, :],
                                     qsc, AF.Identity,
                                     bias=bq_sb[:, m:m + 1])

            # qT layout [dq, hg, j, hh, q]: scores rhs [:, hg, j] is a fully
            # contiguous 512-column block (4 heads x 128 q). Split per 512-q
            # half (j<4 / j>=4) so each half's slot frees right after its
            # last ladder.
            qTs = [
                s32p.tile([128, 4, 4, 4, 128], bf16, tag="s32", bufs=4,
                          name="qT0"),
                s32p.tile([128, 4, 4, 4, 128], bf16, tag="s32", bufs=4,
                          name="qT1"),
            ]

            st = [None] * 6   # 4 full chunks + 2 q-halves
            st[0] = _stats(xc_all[:, 0], "bcr0")
            kv0 = _kv_mains(0)
            st[1] = _stats(xc_all[:, 1], "bcr1")
            _kv_epi(0, *kv0, *st[0])
            kv1 = _kv_mains(1)
            st[2] = _stats(xc_all[:, 2], "bcr2")
            _kv_epi(1, *kv1, *st[1])
            kv2 = _kv_mains(2)
            st[3] = _stats(xc_all[:, 3], "bcr3")
            _kv_epi(2, *kv2, *st[2])
            kv3 = _kv_mains(3)
            st[4] = _stats(xqcs[0], "bcrq0")
            _kv_epi(3, *kv3, *st[3])
            st[5] = _stats(xqcs[1], "bcrq1")
            for m in range(NH):
                _q_head(0, *st[4], m)
            # Q-half1 is emitted interleaved with the short jp0/jp1 attention
            # ladders below (it has no dependency on them) so the PE chews Q
            # matmuls while ACT drains those ladders' exp queue.

            # ---- phase C: attention (scoresT, padded extent E=2j+2) ----
            # Two q-blocks' ladders are interleaved per head-group so the PE
            # fills the ACT-exp latency of one block with the other block's
            # matmuls. attnT is split per 512-q half so phase D's first half
            # can interleave with the last (ACT-heavy) ladders.
            attnTs = [
                s32p.tile([128, NH, 512], bf16, tag="s32", bufs=4,
                          name="attnT0"),
                s32p.tile([128, NH, 512], bf16, tag="s32", bufs=4,
                          name="attnT1"),
            ]

            def _attn_step(hg, j, p0, kts, exacc, av_ps):
                """One 2-kt step of block j: two scores matmuls into a 2-bank
                psum tile, one exp over both, av accumulation on the PE and
                elementwise exp accumulation on the (otherwise idle) DVE —
                the softmax denominator then needs only ONE ones-matmul per
                block instead of one per k-tile. Causal masking is a 0/1
                multiply on exp(s) (exp(s+M) = exp(s)·exp(M)) — on the DVE,
                keeping the PE free of mask matmuls."""
                E = len(kts)
                sc = psA.tile([128, 2, 512], f32, tag="A", bufs=2, name="sc")
                for dk in range(2):
                    kt = kts[p0 + dk]
                    masked = kt >= E - 2
                    nc.tensor.matmul(
                        sc[:, dk, :], kT_sb[:, kt * 128:(kt + 1) * 128],
                        qTs[j // 4][:, hg, j % 4], start=True, stop=not masked,
                    )
                    if masked:
                        # mask add on the PE: sc += I^T @ mask. Keeping this
                        # inside the accumulation group avoids a cross-engine
                        # hop on the exp->AV critical path (measured: a DVE
                        # 0/1-multiply here cost ~1us/block in AV stalls).
                        nc.tensor.matmul(sc[:, dk, :], id_bf,
                                         maskA if kt == E - 2 else maskB,
                                         start=False, stop=True)
                ex = workp.tile([128, 2, 512], bf16, tag="t2k", bufs=2,
                                name="ex")
                nc.scalar.activation(ex, sc, AF.Exp)
                if p0 == 0:
                    nc.vector.tensor_copy(exacc, ex[:, 0, :])
                else:
                    nc.vector.tensor_add(exacc, exacc, ex[:, 0, :])
                nc.vector.tensor_add(exacc, exacc, ex[:, 1, :])
                for dk in range(2):
                    idx = p0 + dk
                    kt = kts[idx]
                    nc.tensor.matmul(av_ps, vtm[:, kt, :], ex[:, dk, :],
                                     start=(idx == 0), stop=(idx == E - 1))

            def _attn_tail(hg, j, exacc, av_ps):
                den_ps = psB.tile([1, 512], f32, tag="B", bufs=2,
                                  name="den_ps")
                nc.tensor.matmul(den_ps, ones_col, exacc, start=True, stop=True)
                lnd = rowp.tile([1, 512], f32, tag="rows", bufs=2, name="lnd")
                nc.scalar.activation(lnd, den_ps, AF.Ln)
                nc.scalar.activation(lnd, lnd, AF.Exp, scale=-1.0)
                rec = lnd
                bcr_ps = psB.tile([128, 512], f32, tag="B", bufs=2,
                                  name="bcr_ps")
                nc.tensor.matmul(bcr_ps, ones_rowf, rec, start=True, stop=True)
                # Stage via SBUF on the DVE (HW: only one non-scalar input
                # may come from PSUM). Short-lived "vfm" tag — the long-lived
                # stats broadcasts (bc1k) must not rotate with these.
                bcr_sb = workp.tile([128, 512], bf16, tag="vfm", bufs=2,
                                    name="bcr_sb")
                nc.vector.tensor_copy(bcr_sb, bcr_ps)
                jj = j % 4
                nc.vector.tensor_mul(
                    attnTs[j // 4][:, hg * 4:(hg + 1) * 4,
                                   jj * 128:(jj + 1) * 128],
                    av_ps, bcr_sb,
                )

            def _ladder(hg, jp):
                j0, j1 = 2 * jp, 2 * jp + 1
                E0, E1 = 2 * j0 + 2, 2 * j1 + 2
                kts0 = [E0 - 2, E0 - 1] + list(range(E0 - 2))
                kts1 = [E1 - 2, E1 - 1] + list(range(E1 - 2))
                ea0 = workp.tile([128, 512], bf16, tag="exac", bufs=2,
                                 name="ea0")
                av0 = psC.tile([128, 512], f32, tag="C", bufs=2, name="av0")
                ea1 = workp.tile([128, 512], bf16, tag="exac", bufs=2,
                                 name="ea1")
                av1 = psC.tile([128, 512], f32, tag="C", bufs=2, name="av1")
                for p0 in range(0, E1, 2):
                    if p0 < E0:
                        _attn_step(hg, j0, p0, kts0, ea0, av0)
                    elif p0 == E0:
                        _attn_tail(hg, j0, ea0, av0)
                    _attn_step(hg, j1, p0, kts1, ea1, av1)
                _attn_tail(hg, j1, ea1, av1)

            # ---- phase D pieces (emitted interleaved with phase C) ----
            ys = [
                s32p.tile([128, HT, 512], bf16, tag="s32", bufs=4, name="y0"),
                None,   # y1 allocated after qT/attnT0 die
            ]

            def _d_ob(ch, ob):
                band = bandp.tile([128, HT, 128], bf16, tag="band4", bufs=3,
                                  name="band")
                nc.sync.dma_start(band, wo_d[ob])
                wps = psA.tile([128, 2, 512], f32, tag="A", bufs=2,
                               name="wps")
                for ht in range(HT):
                    nc.tensor.matmul(wps[:, 0, :], band[:, ht, :],
                                     attnTs[ch][:, ht, :],
                                     start=(ht == 0), stop=(ht == HT - 1))
                xqt = workp.tile([128, 512], bf16, tag="xq1k", bufs=2,
                                 name="xqt")
                nc.sync.dma_start(xqt, xtq_d[ob, ch])
                nc.vector.scalar_tensor_tensor(
                    out=ys[ch][:, ob, :],
                    in0=wps[:, 0, :], scalar=bo_sb[:, ob:ob + 1],
                    in1=xqt, op0=ALU.add, op1=ALU.add,
                )

            # jp0/jp1 ladders interleave with Q-half1 heads; the ACT-heavy
            # jp2/jp3 ladders interleave with phase-D ch0 blocks so the PE
            # chews projection matmuls while ACT drains the exp queue.
            qh1 = iter(range(NH))
            for jp in range(2):
                for hg in range(4):
                    _ladder(hg, jp)
                    _q_head(1, *st[5], next(qh1))
                    _q_head(1, *st[5], next(qh1))
            dch0 = iter(range(HT))
            for jp in (2, 3):
                for hg in range(4):
                    _ladder(hg, jp)
                    _d_ob(0, next(dch0))
                    _d_ob(0, next(dch0))

            def _ln2_acc(ch):
                """LN2 elementwise stats accumulation for one chunk — pure
                ACT/DVE work (two k-tiles per op), emitted right after the
                chunk's y completes so it hides under PE-dense stretches."""
                yc = ys[ch]
                yacc = workp.tile([128, 2, 512], bf16, tag="exac", bufs=2,
                                  name="yacc")
                nc.vector.tensor_add(yacc, yc[:, 0:2, :], yc[:, 2:4, :])
                for kt in range(4, HT, 2):
                    nc.vector.tensor_add(yacc, yacc, yc[:, kt:kt + 2, :])
                nc.vector.tensor_add(yacc[:, 0, :], yacc[:, 0, :],
                                     yacc[:, 1, :])
                sqacc = workp.tile([128, 2, 512], bf16, tag="exac", bufs=2,
                                   name="sqacc")
                nc.scalar.square(sqacc, yc[:, 0:2, :])
                for kt in range(2, HT, 2):
                    sq = workp.tile([128, 2, 512], bf16, tag="t2k", bufs=2,
                                    name="sq2")
                    nc.scalar.square(sq, yc[:, kt:kt + 2, :])
                    nc.vector.tensor_add(sqacc, sqacc, sq)
                nc.vector.tensor_add(sqacc[:, 0, :], sqacc[:, 0, :],
                                     sqacc[:, 1, :])
                return yacc, sqacc

            def _ln2_fin_rows(yacc, sqacc):
                """LN2 reduction matmuls + row chain."""
                sum_ps = psB.tile([1, 512], f32, tag="B", bufs=2, name="l2sum")
                nc.tensor.matmul(sum_ps, ones_col, yacc[:, 0, :], start=True,
                                 stop=True)
                sumsq_ps = psC.tile([1, 512], f32, tag="C", bufs=2, name="l2sq")
                nc.tensor.matmul(sumsq_ps, ones_col, sqacc[:, 0, :],
                                 start=True, stop=True)
                return _ln_rows(nc, rowp, workp, psB, sum_ps, sumsq_ps, eps_t)

            def _ln2_fin_norm(ch, m_sb, rstd):
                """LN2 broadcast + normalize."""
                yc = ys[ch]
                bcm = _bcast_row_bf16(nc, workp, psC, ones_rowf, m_sb, "l2bcm")
                bcr = _bcast_row_bf16(nc, workp, psC, ones_rowf, rstd, "l2bcr")
                ln2s = s32p.tile([128, HT, 512], bf16, tag="s32", bufs=4,
                                 name="ln2s")
                for kt in range(HT):
                    nc.vector.tensor_sub(ln2s[:, kt, :], yc[:, kt, :], bcm)
                    nc.vector.tensor_mul(ln2s[:, kt, :], ln2s[:, kt, :], bcr)
                return ln2s

            def _ln2_fin(ch, yacc, sqacc):
                return _ln2_fin_norm(ch, *_ln2_fin_rows(yacc, sqacc))

            acc0 = _ln2_acc(0)
            ys[1] = s32p.tile([128, HT, 512], bf16, tag="s32", bufs=4,
                              name="y1")
            # fin0's pieces slot between early D-ch1 blocks so its chain
            # never exposes on the PE: stats matmuls after 2 blocks (acc0's
            # chains finish under them), broadcasts 2 blocks later (the row
            # chain finishes under those), normalize runs under the rest.
            _d_ob(1, 0)
            _d_ob(1, 1)
            rows0 = _ln2_fin_rows(*acc0)
            _d_ob(1, 2)
            _d_ob(1, 3)
            ln2s0 = _ln2_fin_norm(0, *rows0)
            for ob in range(4, HT):
                _d_ob(1, ob)

            # ---- phase E: LN2 + MLP + residual -> out (per 512-token chunk)
            # Emission order keeps each chunk's stats chain off the critical
            # path: fin0 right after D (acc0 ran under D-ch1), acc1 under
            # fc-ch0, fin1 between fc-ch0 and proj-ch0.
            ln2s_pair = [ln2s0, None]
            acc1 = _ln2_acc(1)
            for ch in range(2):
                cols = slice(ch * 512, (ch + 1) * 512)
                ln2s = ln2s_pair[ch]

                gT = bigp.tile([128, IT, 512], bf16, tag="big", name="gT")
                for mb in range(IT):
                    band = bandp.tile([128, HT, 128], bf16, tag="band4",
                                      bufs=3, name="band")
                    nc.sync.dma_start(band, wfc_d[mb])
                    fps = psA.tile([128, 2, 512], f32, tag="A", bufs=2,
                                   name="fps")
                    for kt in range(HT):
                        nc.tensor.matmul(fps[:, 0, :], band[:, kt, :],
                                         ln2s[:, kt, :],
                                         start=(kt == 0), stop=(kt == HT - 1))
                    nc.scalar.activation(gT[:, mb, :], fps[:, 0, :],
                                         AF.Gelu_apprx_tanh,
                                         bias=bfc_sb[:, mb:mb + 1])
                if ch == 0:
                    ln2s_pair[1] = _ln2_fin(1, *acc1)

                for ob in range(HT):
                    pband = strp.tile([128, IT, 128], bf16, tag="str16",
                                      name="pband")
                    nc.sync.dma_start(pband, wproj_d[ob])
                    pps = psA.tile([128, 2, 512], f32, tag="A", bufs=2,
                                   name="pps")
                    for mt in range(IT):
                        nc.tensor.matmul(pps[:, 0, :], pband[:, mt, :],
                                         gT[:, mt, :],
                                         start=(mt == 0), stop=(mt == IT - 1))
                    osb = workp.tile([128, 512], bf16, tag="f2k", bufs=2,
                                     name="osb")
                    nc.vector.scalar_tensor_tensor(
                        out=osb, in0=pps[:, 0, :],
                        scalar=bproj_sb[:, ob:ob + 1],
                        in1=ys[ch][:, ob, :], op0=ALU.add, op1=ALU.add,
                    )
                    nc.sync.dma_start(
                        out_d[ob * 128:(ob + 1) * 128, cols], osb
                    )
    _split_excess_waits(nc)
    return nc


_PROG = None


def _get_prog():
    global _PROG
    if _PROG is None:
        _PROG = _build_program()
    return _PROG


def _to_bf(a):
    return np.ascontiguousarray(a.astype(ml_dtypes.bfloat16))


def kernel(hidden_states, ln1_g, ln1_b, ln2_g, ln2_b, wq, bq, wkv, bkv,
           wo, bo, wfc, bfc, wproj, bproj):
    hs = np.asarray(hidden_states, np.float32)
    ln1_g = np.asarray(ln1_g, np.float32)
    ln1_b = np.asarray(ln1_b, np.float32)
    ln2_g = np.asarray(ln2_g, np.float32)
    ln2_b = np.asarray(ln2_b, np.float32)
    wq = np.asarray(wq, np.float32)
    wkv = np.asarray(wkv, np.float32)
    wo = np.asarray(wo, np.float32)
    wfc = np.asarray(wfc, np.float32)
    wproj = np.asarray(wproj, np.float32)

    # Fold LN gains into the following matmuls; fold qk scale into K.
    wq_e = ln1_g[:, None] * wq
    bq_e = np.asarray(bq, np.float32) + ln1_b @ wq
    wkv_e = ln1_g[:, None] * wkv
    bkv_e = np.asarray(bkv, np.float32) + ln1_b @ wkv
    scale = 1.0 / np.sqrt(D)
    wk_e = wkv_e[:, :D] * scale
    bk_e = bkv_e[:D] * scale
    wv_e = wkv_e[:, D:]
    bv_e = bkv_e[D:]
    wfc_e = ln2_g[:, None] * wfc
    bfc_e = np.asarray(bfc, np.float32) + ln2_b @ wfc

    # Host-packed weight layouts: [out-block, partition, k-tile, n] so each
    # band DMA is contiguous per partition line.
    wq_l = _to_bf(wq_e.reshape(HT, 128, NH, 128).transpose(2, 1, 0, 3))
    wk_l = _to_bf(wk_e.reshape(HT, 128, 128).transpose(1, 0, 2))
    wv_l = _to_bf(wv_e.reshape(HT, 128, 128).transpose(1, 0, 2))
    wo_l = _to_bf(wo.reshape(HT, 128, HT, 128).transpose(2, 1, 0, 3))
    wfc_l = _to_bf(wfc_e.reshape(HT, 128, IT, 128).transpose(2, 1, 0, 3))
    wproj_l = _to_bf(wproj.reshape(IT, 128, HT, 128).transpose(2, 1, 0, 3))

    # Negated column sums for the K=1 LN-mean-fold correction matmuls.
    csq_r = _to_bf(-wq_e.sum(axis=0).reshape(1, NH, 128))
    csk_r = _to_bf(-wk_e.sum(axis=0)[None, :])
    csv_r = _to_bf(-wv_e.sum(axis=0)[None, :])

    bq_r = np.ascontiguousarray(bq_e.reshape(NH, 128).T)
    # bk is dropped on-device (a per-(head,q) constant in the logits is
    # softmax-invariant); bv folds into bo exactly (probs sum to 1).
    bo_e = np.asarray(bo, np.float32) + np.tile(bv_e, NH) @ wo
    bo_r = np.ascontiguousarray(bo_e.reshape(HT, 128).T)
    bfc_r = np.ascontiguousarray(bfc_e.reshape(IT, 128).T)
    bproj_r = np.ascontiguousarray(
        np.asarray(bproj, np.float32).reshape(HT, 128).T)

    # Causal masks for the two parity-dependent diagonal k-tiles.
    tri = np.where(np.arange(128)[None, :] >= np.arange(128)[:, None],
                   0.0, NEG).astype(np.float32)          # [k,q]
    tri4 = np.tile(tri, (1, 4))                          # [128, 512] (4 heads)
    zeros4 = np.zeros((128, 512), np.float32)
    neg4 = np.full((128, 512), NEG, np.float32)
    mask_h = [(_to_bf(tri4), _to_bf(neg4)),              # parity 0: (A, B)
              (_to_bf(zeros4), _to_bf(tri4))]            # parity 1: (A, B)

    in_maps = []
    gmaps = []
    for c in range(8):
        b, h = divmod(c, 2)
        gmap = [2 * j + h for j in range(8)]
        gmaps.append(gmap)
        xb = hs[b]                                        # [2048, 2048]
        xt_h = _to_bf(xb.reshape(NCH, 512, HT, 128).transpose(0, 3, 2, 1))
        xqb = xb.reshape(16, 128, H)[gmap].reshape(NQ, H)  # [1024, 2048]
        xq_h = _to_bf(xqb.reshape(2, 512, HT, 128).transpose(0, 3, 2, 1))
        xtq_h = _to_bf(xqb.reshape(2, 512, HT, 128).transpose(2, 0, 3, 1))
        mA, mB = mask_h[h]
        in_maps.append(dict(
            xt=xt_h, xq=xq_h, xtq=xtq_h,
            wq=wq_l, wk=wk_l, wv=wv_l, wo=wo_l, wfc=wfc_l, wproj=wproj_l,
            bq=bq_r, bo=bo_r, bfc=bfc_r, bproj=bproj_r,
            maskA=mA, maskB=mB, csq=csq_r, csk=csk_r, csv=csv_r,
        ))

    res = run_bass_kernel_spmd(_get_prog(), in_maps, core_ids=list(range(8)))
    kernel.last_result = res

    out = np.empty((B, S, H), np.float32)
    for c in range(8):
        b, h = divmod(c, 2)
        resT = np.asarray(res.results[c]["out"]).astype(np.float32)
        blocks = resT.T.reshape(8, 128, H)                # local q-blocks
        for j, g in enumerate(gmaps[c]):
            out[b, g * 128:(g + 1) * 128, :] = blocks[j]
    return out


kernel.last_result = None

